# revision 26
# baseline (speedup 1.0000x reference)
"""Trainium2 Bass kernel for nn_AttentionModulatedOrdinalEmbedding.

Contract: kernel(**inputs) takes the FULL (unsharded) inputs from
setup_inputs() and returns the FULL (B, S, EMB) float32 output.
Internally shards batch-parallel across 8 NeuronCores (4 batches/core),
runs one SPMD Bass kernel, and concatenates the per-core outputs.

Hardcoded problem shape: B=32, S=512, N_Q=1024, N_CATS=4, EMB=64,
ATTN=32, HEADS=4 (head_dim 8).

Device-side structure (per core, 4 batches of 512 tokens):
- All weight algebra is folded on the host: ctx projection into q/k/v
  (with biases via an appended ones row of ceT), W_out into W_sup, and
  the ordinal-softmax path into a per-token (128,64) `sharp` table.
- The exp over the (4 heads x 512 x 512) score matrix per batch is the
  ACT-engine bottleneck; everything else is scheduled around keeping
  that stream dense: PE runs one wave ahead, DVE/Pool do evacuations
  and the final gather-contract, DMAs are front-loaded on the SP queue.
"""

import os
import sys
from contextlib import ExitStack

import numpy as np

for _p in ("/opt/trn_rl_repo", "/root/.axon_site/_ro/trn_rl_repo"):
    if os.path.isdir(_p) and _p not in sys.path:
        sys.path.append(_p)

import ml_dtypes  # noqa: E402

import concourse.bass as bass  # noqa: E402
import concourse.tile as tile  # noqa: E402
from concourse import bacc, mybir  # noqa: E402
from concourse.bass import IndirectOffsetOnAxis  # noqa: E402
from concourse.bass_utils import run_bass_kernel_spmd  # noqa: E402

BF16 = ml_dtypes.bfloat16
F32 = mybir.dt.float32
BF = mybir.dt.bfloat16
I32 = mybir.dt.int32
ALU = mybir.AluOpType
ACTF = mybir.ActivationFunctionType

B, S, EMB, ATTN, HEADS, HD, C, Q = 32, 512, 64, 32, 4, 8, 4, 1024
NCORES = 8
NB = B // NCORES          # batches per core = 4
NJ = NB * (S // 128)      # token tiles per core = 16
SCALE = 1.0 / np.sqrt(HD)
WARMUP = 18               # PE warm matmuls to trip the HAM clock boost
MERGED_QK = os.environ.get("K_MERGED_QK", "1") == "1"
ACT_Q_DMA = os.environ.get("K_ACT_DMA", "1") == "1"


def build_kernel(nc: bacc.Bacc, tc: tile.TileContext, io: dict, skip_ebc: bool):
    ctx = ExitStack()
    with ctx, nc.allow_low_precision(reason="bf16 accum within tolerance"):
        _build(nc, tc, ctx, io, skip_ebc)


def _build(nc, tc, ctx, io, skip_ebc):
    const = ctx.enter_context(tc.tile_pool(name="const", bufs=1))
    sb = ctx.enter_context(tc.tile_pool(name="sb", bufs=2))
    expp = ctx.enter_context(tc.tile_pool(name="expp", bufs=32))
    big = ctx.enter_context(tc.tile_pool(name="big", bufs=1))
    ps_scores = ctx.enter_context(tc.tile_pool(name="ps_scores", bufs=2, space="PSUM"))
    ps_av = ctx.enter_context(tc.tile_pool(name="ps_av", bufs=1, space="PSUM"))
    ps_sum = ctx.enter_context(tc.tile_pool(name="ps_sum", bufs=1, space="PSUM"))
    ps_misc = ctx.enter_context(tc.tile_pool(name="ps_misc", bufs=2, space="PSUM"))

    # ---------------- DMA loads: ramp-critical ceT0+wqkv lead the SP
    # queue; qidx goes on the ACT queue (hwdge-capable, idle during ramp).
    cet = {}
    cet[0] = const.tile([EMB + 1, S], BF, tag="cet0", name="cet0")
    nc.sync.dma_start(out=cet[0][:, :], in_=io["ce"][0, :, :])

    wqkv = const.tile([EMB + 1, 3 * 128], BF, tag="wqkv")
    nc.sync.dma_start(out=wqkv[:, :], in_=io["wqkv"][:, :])

    dma_q2 = nc.scalar if ACT_Q_DMA else nc.sync
    qidx = const.tile([128, NJ], I32, tag="qidx")
    dma_q2.dma_start(out=qidx[:, :], in_=io["qidx"][:, :])

    # blob: sharp (0:64) | wzs_spT (64:68) | E_bc (68:84), all bf16
    blob = const.tile([128, 84], BF, tag="blob")
    nc.sync.dma_start(out=blob[:, :], in_=io["blob"][:, :])
    sharp = blob[:, 0:64]
    wzs = blob[:, 64:68]
    ebc = blob[:, 68:84]

    for b in range(1, NB):
        cet[b] = const.tile([EMB + 1, S], BF, tag=f"cet{b}", name=f"cet{b}")
        nc.sync.dma_start(out=cet[b][:, :], in_=io["ce"][b, :, :])

    # ---------------- small constants on idle engines --------------------
    ones_bf = const.tile([128, ATTN], BF, tag="ones_bf")
    nc.gpsimd.memset(ones_bf[:, :], 1.0)
    warm = const.tile([128, 128], BF, tag="warm")
    nc.vector.memset(warm[:, :], 0.5)

    # PE warm-up: dense matmuls at kernel start trip the HAM activity
    # window so the whole kernel runs at 2.4 GHz instead of 1.2.
    warm_ps = ps_misc.tile([128, 128], F32, tag="misc", name="warm_ps")
    for _ in range(WARMUP):
        nc.tensor.matmul(warm_ps[0:32, :], warm[:, 0:32], warm[:, :],
                         start=True, stop=True)

    # ---------------- gathers (gpsimd queue; independent of attention) ---
    # g_all free layout: j (16) x e (64) x c (4); one indirect DMA per j.
    g_all = big.tile([128, NJ * C * EMB], BF, tag="g_all")
    for j in range(NJ):
        nc.gpsimd.indirect_dma_start(
            out=g_all[:, C * EMB * j : C * EMB * (j + 1)],
            out_offset=None,
            in_=io["w3T"][:, :],
            in_offset=IndirectOffsetOnAxis(ap=qidx[:, j : j + 1], axis=0),
        )

    fw = big.tile([128, NJ * C], BF, tag="fw")
    out_all = big.tile([128, NJ * EMB], BF, tag="out_all")

    # ---------------- per-batch stages ------------------------------------
    qs_l, ks_l, v_l = {}, {}, {}

    def stage_qk(b):
        ceT = cet[b]
        qs_ps = ps_misc.tile([128, S], F32, tag="misc", name="qs_ps")
        for h in range(HEADS):
            nc.tensor.matmul(
                qs_ps[32 * h : 32 * (h + 1), :],
                wqkv[:, 32 * h : 32 * (h + 1)],
                ceT[:, :],
                start=True, stop=True,
                tile_position=(0, 32 * h),
            )
        qs = sb.tile([128, S], BF, tag="qs", name="qs")
        nc.vector.tensor_copy(qs[:, :], qs_ps[:, :])
        qs_l[b] = qs
        ks_ps = ps_misc.tile([128, S], F32, tag="misc", name="ks_ps")
        for h in range(HEADS):
            nc.tensor.matmul(
                ks_ps[32 * h : 32 * (h + 1), :],
                wqkv[:, 128 + 32 * h : 128 + 32 * (h + 1)],
                ceT[:, :],
                start=True, stop=True,
                tile_position=(0, 32 * h),
            )
        ks = sb.tile([128, S], BF, tag="ks", name="ks")
        # b=0 is ramp-critical: evac ks on the (still idle) ACT engine so the
        # qs/ks casts run in parallel; copy shares the Exp activation table.
        if b == 0:
            nc.scalar.copy(ks[:, :], ks_ps[:, :])
        else:
            nc.vector.tensor_copy(ks[:, :], ks_ps[:, :])
        ks_l[b] = ks

    def stage_v(b):
        ceT = cet[b]
        v_ps = ps_misc.tile([128, S], F32, tag="misc", name="v_ps")
        for cc in range(4):
            nc.tensor.matmul(
                v_ps[:, 128 * cc : 128 * (cc + 1)],
                ceT[:, 128 * cc : 128 * (cc + 1)],
                wqkv[:, 256:384],
                start=True, stop=True,
            )
        v_sp = sb.tile([128, S], BF, tag="v_sp", name="v_sp")
        nc.vector.tensor_copy(v_sp[:, :], v_ps[:, :])
        v_l[b] = v_sp

    # ---- scores/exp/AV software pipeline --------------------------------
    def qk_wave(b, cc):
        qs, ks = qs_l[b], ks_l[b]
        ets = []
        tiles = []
        for hh in range(2):  # head halves
            sc_ps = ps_scores.tile([128, 2 * S], F32, tag="scores")
            for hi in range(2):
                h = 2 * hh + hi
                if MERGED_QK:
                    nc.tensor.matmul(
                        sc_ps[:, S * hi : S * (hi + 1)],
                        ks[32 * h : 32 * h + HD, 128 * cc : 128 * (cc + 1)],
                        qs[32 * h : 32 * h + HD, :],
                        start=True,
                        stop=True,
                        tile_position=(32 * h, 0),
                    )
                else:
                    for jj in range(4):
                        nc.tensor.matmul(
                            sc_ps[32 * jj : 32 * (jj + 1), S * hi : S * (hi + 1)],
                            ks[32 * h : 32 * h + HD,
                               128 * cc + 32 * jj : 128 * cc + 32 * (jj + 1)],
                            qs[32 * h : 32 * h + HD, :],
                            start=True,
                            stop=True,
                            tile_position=(32 * h, 32 * jj),
                        )
            tiles.append(sc_ps)
        for sc_ps in tiles:
            et = expp.tile([128, 2 * S], BF, tag="expT")
            nc.scalar.activation(et[:, :], sc_ps[:, :], ACTF.Exp, scale=SCALE)
            ets.append(et)
        return ets

    def av_wave(b, cc, avt_ps, sums_ps, ets):
        # emitted per head-half so each half only depends on its own exp
        # tile; sums before AV so the last batch's reciprocal starts sooner.
        for hh in range(2):
            for hi in range(2):
                h = 2 * hh + hi
                mv = ets[hh][:, S * hi : S * (hi + 1)]
                nc.tensor.matmul(
                    sums_ps[32 * h : 32 * (h + 1), :],
                    ones_bf[:, :],
                    mv,
                    start=(cc == 0),
                    stop=(cc == 3),
                    tile_position=(0, 32 * h),
                    skip_group_check=True,
                )
            for hi in range(2):
                h = 2 * hh + hi
                mv = ets[hh][:, S * hi : S * (hi + 1)]
                nc.tensor.matmul(
                    avt_ps[32 * h : 32 * (h + 1), :],
                    v_l[b][:, 128 * cc + 32 * h : 128 * cc + 32 * (h + 1)],
                    mv,
                    start=(cc == 0),
                    stop=(cc == 3),
                    tile_position=(0, 32 * h),
                    skip_group_check=True,
                )

    def post_batch(b, avt_ps, sums_ps):
        rec = sb.tile([128, S], F32, tag="rec")
        nc.vector.reciprocal_approx_fast(rec[:, :], sums_ps[:, :])
        # normT in halves so the z matmuls for cc 0-1 start while the
        # second half normalizes (suppression chain is the serial tail).
        normT = sb.tile([128, S], BF, tag="normT")
        z_ps = ps_misc.tile([128, 4 * C], F32, tag="misc", name="z_ps")
        for half in range(2):
            sl = slice(256 * half, 256 * (half + 1))
            nc.vector.tensor_tensor(
                normT[:, sl], avt_ps[:, sl], rec[:, sl], op=ALU.mult
            )
            for cc in (2 * half, 2 * half + 1):
                nc.tensor.matmul(
                    z_ps[:, C * cc : C * (cc + 1)],
                    normT[:, 128 * cc : 128 * (cc + 1)],
                    wzs[:, :],
                    start=True,
                    stop=True,
                )
        # u = 2 - sigmoid(z+bz) = (2+t)/(1+t) = 1 + 1/(1+t), t = e^z * e^bz
        tb = sb.tile([128, 4 * C], F32, tag="tb")
        nc.scalar.activation(tb[:, :], z_ps[:, :], ACTF.Exp)
        if not skip_ebc:
            tbe = sb.tile([128, 4 * C], F32, tag="tbe")
            nc.vector.tensor_tensor(tbe[:, :], tb[:, :], ebc[:, :], op=ALU.mult)
            tb = tbe
        ab = sb.tile([128, 4 * C], F32, tag="ab")
        nc.vector.tensor_scalar_add(ab[:, :], tb[:, :], 1.0)
        rb = sb.tile([128, 4 * C], F32, tag="rb")
        nc.vector.reciprocal_approx_fast(rb[:, :], ab[:, :])
        # fw = u * sharp = (1 + rb) * sharp  (0.5/head-mean folded in sharp)
        nc.vector.scalar_tensor_tensor(
            fw[:, 16 * b : 16 * (b + 1)],
            rb[:, :],
            1.0,
            sharp[:, 16 * b : 16 * (b + 1)],
            op0=ALU.add,
            op1=ALU.mult,
        )

        # final gather-contract: out[j,e] = sum_c g[j,e,c] * fw[j,c]
        # split across DVE and Pool for the last batch to shorten the tail.
        def gview(j0, j1):
            gsl = g_all[:, C * EMB * (NB * b + j0) : C * EMB * (NB * b + j1)]
            gv = gsl.rearrange("p (j e c) -> p j e c", e=EMB, c=C)
            fv = fw[:, 16 * b + C * j0 : 16 * b + C * j1].rearrange(
                "p (j c) -> p j c", c=C
            )[:, :, None, :].to_broadcast([128, j1 - j0, EMB, C])
            return gv, fv

        def reduce_half(j0, j1):
            gv, _ = gview(j0, j1)
            osl = out_all[:, EMB * (NB * b + j0) : EMB * (NB * b + j1)]
            nc.vector.tensor_reduce(
                osl.rearrange("p (j e) -> p j e", e=EMB),
                gv,
                axis=mybir.AxisListType.X,
                op=ALU.add,
            )

        def store(j0, j1):
            nc.sync.dma_start(
                out=io["out"][:, NB * b + j0 : NB * b + j1, :],
                in_=out_all[
                    :, EMB * (NB * b + j0) : EMB * (NB * b + j1)
                ].rearrange("p (j e) -> p j e", e=EMB),
            )

        if b == NB - 1:
            gv0, fv0 = gview(0, 2)
            nc.vector.tensor_tensor(gv0, gv0, fv0, op=ALU.mult)
            gv1, fv1 = gview(2, 4)
            nc.gpsimd.tensor_tensor(gv1, gv1, fv1, op=ALU.mult)
            reduce_half(0, 2)
            store(0, 2)
            reduce_half(2, 4)
            store(2, 4)
        else:
            gv0, fv0 = gview(0, 4)
            nc.vector.tensor_tensor(gv0, gv0, fv0, op=ALU.mult)
            reduce_half(0, 4)
            store(0, 4)

    ets_l = {b: [None] * 4 for b in range(NB)}
    av_tiles = {}

    def get_av(b):
        if b not in av_tiles:
            av_tiles[b] = (
                ps_av.tile([128, S], F32, tag="avt", name="avt_ps"),
                ps_sum.tile([128, S], F32, tag="sums", name="sums_ps"),
            )
        return av_tiles[b]

    stage_qk(0)
    prev = None  # (b, cc) whose AV wave is pending, 1 step behind
    for b in range(NB):
        for cc in range(4):
            ets_l[b][cc] = qk_wave(b, cc)
            if b == 0 and cc == 0:
                stage_v(0)
            if b + 1 < NB:
                # staged mid-batch: the boundary waves already carry the
                # previous batch's AV drain + post work on the PE queue.
                if cc == 1:
                    stage_qk(b + 1)
                elif cc == 2:
                    stage_v(b + 1)
            if prev is not None:
                pb, pcc = prev
                av_wave(pb, pcc, *get_av(pb), ets_l[pb][pcc])
                if pcc == 3:
                    post_batch(pb, *av_tiles[pb])
            prev = (b, cc)
    av_wave(NB - 1, 3, *get_av(NB - 1), ets_l[NB - 1][3])
    post_batch(NB - 1, *av_tiles[NB - 1])


# ======================= host side =======================

def _prep_weights(inp):
    """Pure host-side folding of the (small, replicated) parameters."""
    f32 = np.float32

    def bf(x):
        return np.ascontiguousarray(np.asarray(x, f32).astype(BF16))

    W_ctx = np.asarray(inp["W_ctx"], f32)
    W_in = np.asarray(inp["W_in"], f32)
    W_out = np.asarray(inp["W_out"], f32)
    W_sup = np.asarray(inp["W_sup"], f32)
    W_emb = np.asarray(inp["W_emb"], f32)
    b_ctx = np.asarray(inp["b_ctx"], f32)
    b_in = np.asarray(inp["b_in"], f32)
    b_out = np.asarray(inp["b_out"], f32)
    b_sup = np.asarray(inp["b_sup"], f32)
    temp = np.asarray(inp["temperature"], np.float64)

    w = {}

    # q/k/v projections folded with the ctx projection; bias via ones row.
    def spread(M, c):
        # M (32, 64) rows 8h+d, c (32,) -> (65, 128) cols 32h+d
        out = np.zeros((EMB + 1, 128), f32)
        for h in range(HEADS):
            out[0:EMB, 32 * h : 32 * h + HD] = M[HD * h : HD * (h + 1), :].T
            out[EMB, 32 * h : 32 * h + HD] = c[HD * h : HD * (h + 1)]
        return out

    parts = []
    for i in range(3):
        Wp = W_in[ATTN * i : ATTN * (i + 1)]
        bp = b_in[ATTN * i : ATTN * (i + 1)]
        parts.append(spread(Wp @ W_ctx, Wp @ b_ctx + bp))
    w["wqkv"] = bf(np.concatenate(parts, axis=1))  # (65, 384)

    # suppression: z = o @ (W_sup W_out)^T + (W_sup b_out + b_sup)
    Wz = W_sup @ W_out            # (4, 32)
    bz = W_sup @ b_out + b_sup    # (4,)
    wzs = np.zeros((128, C), f32)
    for h in range(HEADS):
        wzs[32 * h : 32 * h + HD, :] = Wz[:, HD * h : HD * (h + 1)].T
    ebc = np.broadcast_to(np.tile(np.exp(bz).astype(f32), 4)[None, :], (128, 16))

    # ordinal-softmax table: tbl[r, c] = 0.5 * mean_h softmax_c(bw(r,.)/T_h)
    kk = np.arange(C, dtype=np.float64)
    tbl = np.zeros((C, C), np.float64)
    for r in range(C):
        bw = np.clip(1.0 - np.abs(kk - r) / (C - 1), 0.0, None)
        sh = np.exp(bw[None, :] / temp[:, None])
        sh /= sh.sum(axis=1, keepdims=True)
        tbl[r] = 0.5 * sh.mean(axis=0)
    w["_tbl"] = tbl.astype(f32)
    w["_wzs"] = wzs
    w["_ebc"] = np.ascontiguousarray(ebc, dtype=f32)

    # gather table: w3T[q, 4e+c] = W_emb[e, c*Q+q]
    w["w3T"] = bf(W_emb.reshape(EMB, C, Q).transpose(2, 0, 1).reshape(Q, EMB * C))
    return w


def _spec():
    return {
        "wqkv": ((EMB + 1, 3 * 128), BF),
        "blob": ((128, 84), BF),
        "qidx": ((128, NJ), I32),
        "w3T": ((Q, C * EMB), BF),
        "ce": ((NB, EMB + 1, S), BF),
    }


def build_bass(skip_ebc: bool = False):
    nc = bacc.Bacc("TRN2", target_bir_lowering=False, debug=False)
    io = {}
    for name, (shape, dt) in _spec().items():
        io[name] = nc.dram_tensor(name, list(shape), dt, kind="ExternalInput").ap()
    io["out"] = nc.dram_tensor("out", [128, NJ, EMB], BF, kind="ExternalOutput").ap()
    with tile.TileContext(nc) as tc:
        build_kernel(nc, tc, io, skip_ebc)
    nc.compile()
    return nc


def skip_ebc_for(inputs) -> bool:
    """True when W_sup@b_out + b_sup == 0, so e^bz == 1 can be elided."""
    W_sup = np.asarray(inputs["W_sup"], np.float64)
    b_out = np.asarray(inputs["b_out"], np.float64)
    b_sup = np.asarray(inputs["b_sup"], np.float64)
    return bool(np.all(W_sup @ b_out + b_sup == 0.0))


def make_in_maps(inputs):
    inp = dict(inputs)
    w = _prep_weights(inp)
    tbl, wzs, ebc = w.pop("_tbl"), w.pop("_wzs"), w.pop("_ebc")
    q_idx = np.asarray(inp["q_idx"]).astype(np.int32)
    r_data = np.asarray(inp["r_data"]).astype(np.int64)
    ce = np.asarray(inp["context_embedding"], np.float32)

    in_maps = []
    for k in range(NCORES):
        m = dict(w)
        qs = q_idx[NB * k : NB * (k + 1)]          # (4,512)
        rs = r_data[NB * k : NB * (k + 1)]
        # token-tile layout: [p, j] with j = 4*b + cc, s = 128*cc + p
        m["qidx"] = np.ascontiguousarray(
            qs.reshape(NB, 4, 128).transpose(2, 0, 1).reshape(128, NJ)
        )
        # sharp values per token: (128, 64) = (p, j*4+c)
        sharp = tbl[rs.reshape(NB, 4, 128)]        # (b, cc, p, c)
        sharp = sharp.transpose(2, 0, 1, 3).reshape(128, NJ * C)
        blob = np.zeros((128, 84), np.float32)
        blob[:, 0:64] = sharp
        blob[:, 64:68] = wzs
        blob[:, 68:84] = ebc
        m["blob"] = blob.astype(BF16)
        # ceT with ones row: (NB, 65, 512)
        cek = ce[NB * k : NB * (k + 1)]            # (4, 512, 64)
        cet = np.ones((NB, EMB + 1, S), np.float32)
        cet[:, 0:EMB, :] = cek.transpose(0, 2, 1)
        m["ce"] = cet.astype(BF16)
        in_maps.append(m)
    return in_maps


_NC_CACHE = {}


def postprocess(res, inputs) -> np.ndarray:
    b_emb = np.asarray(inputs["b_emb"], np.float32)
    outs = []
    for k in range(NCORES):
        o = np.asarray(res.results[k]["out"]).astype(np.float32)  # (128,16,64)
        o = o.reshape(128, NB, 4, EMB).transpose(1, 2, 0, 3).reshape(NB, S, EMB)
        outs.append(o)
    out = np.concatenate(outs, axis=0) + b_emb[None, None, :]
    return out.astype(np.float32)


def kernel(**inputs) -> np.ndarray:
    key = skip_ebc_for(inputs)
    if key not in _NC_CACHE:
        _NC_CACHE[key] = build_bass(skip_ebc=key)
    nc = _NC_CACHE[key]
    in_maps = make_in_maps(inputs)
    res = run_bass_kernel_spmd(nc, in_maps, core_ids=list(range(NCORES)))
    return postprocess(res, inputs)


# revision 27
# speedup vs baseline: 1.0021x; 1.0021x over previous
"""Trainium2 Bass kernel for nn_AttentionModulatedOrdinalEmbedding.

Contract: kernel(**inputs) takes the FULL (unsharded) inputs from
setup_inputs() and returns the FULL (B, S, EMB) float32 output.
Internally shards batch-parallel across 8 NeuronCores (4 batches/core),
runs one SPMD Bass kernel, and concatenates the per-core outputs.

Hardcoded problem shape: B=32, S=512, N_Q=1024, N_CATS=4, EMB=64,
ATTN=32, HEADS=4 (head_dim 8).

Device-side structure (per core, 4 batches of 512 tokens):
- All weight algebra is folded on the host: ctx projection into q/k/v
  (with biases via an appended ones row of ceT), W_out into W_sup, and
  the ordinal-softmax path into a per-token (128,64) `sharp` table.
- The exp over the (4 heads x 512 x 512) score matrix per batch is the
  ACT-engine bottleneck; everything else is scheduled around keeping
  that stream dense: PE runs one wave ahead, DVE/Pool do evacuations
  and the final gather-contract, DMAs are front-loaded on the SP queue.
"""

import os
import sys
from contextlib import ExitStack

import numpy as np

for _p in ("/opt/trn_rl_repo", "/root/.axon_site/_ro/trn_rl_repo"):
    if os.path.isdir(_p) and _p not in sys.path:
        sys.path.append(_p)

import ml_dtypes  # noqa: E402

import concourse.bass as bass  # noqa: E402
import concourse.tile as tile  # noqa: E402
from concourse import bacc, mybir  # noqa: E402
from concourse.bass import IndirectOffsetOnAxis  # noqa: E402
from concourse.bass_utils import run_bass_kernel_spmd  # noqa: E402

BF16 = ml_dtypes.bfloat16
F32 = mybir.dt.float32
BF = mybir.dt.bfloat16
I32 = mybir.dt.int32
ALU = mybir.AluOpType
ACTF = mybir.ActivationFunctionType

B, S, EMB, ATTN, HEADS, HD, C, Q = 32, 512, 64, 32, 4, 8, 4, 1024
NCORES = 8
NB = B // NCORES          # batches per core = 4
NJ = NB * (S // 128)      # token tiles per core = 16
SCALE = 1.0 / np.sqrt(HD)
WARMUP = 18               # PE warm matmuls to trip the HAM clock boost
MERGED_QK = os.environ.get("K_MERGED_QK", "1") == "1"
ACT_Q_DMA = os.environ.get("K_ACT_DMA", "1") == "1"


def build_kernel(nc: bacc.Bacc, tc: tile.TileContext, io: dict, skip_ebc: bool):
    ctx = ExitStack()
    with ctx, nc.allow_low_precision(reason="bf16 accum within tolerance"):
        _build(nc, tc, ctx, io, skip_ebc)


def _build(nc, tc, ctx, io, skip_ebc):
    const = ctx.enter_context(tc.tile_pool(name="const", bufs=1))
    sb = ctx.enter_context(tc.tile_pool(name="sb", bufs=2))
    expp = ctx.enter_context(tc.tile_pool(name="expp", bufs=32))
    big = ctx.enter_context(tc.tile_pool(name="big", bufs=1))
    ps_scores = ctx.enter_context(tc.tile_pool(name="ps_scores", bufs=2, space="PSUM"))
    ps_av = ctx.enter_context(tc.tile_pool(name="ps_av", bufs=1, space="PSUM"))
    ps_sum = ctx.enter_context(tc.tile_pool(name="ps_sum", bufs=1, space="PSUM"))
    ps_misc = ctx.enter_context(tc.tile_pool(name="ps_misc", bufs=2, space="PSUM"))

    # ---------------- DMA loads: ramp-critical ceT0+wqkv lead the SP
    # queue; qidx goes on the ACT queue (hwdge-capable, idle during ramp).
    cet = {}
    cet[0] = const.tile([EMB + 1, S], BF, tag="cet0", name="cet0")
    nc.sync.dma_start(out=cet[0][:, :], in_=io["ce"][0, :, :])

    wqkv = const.tile([EMB + 1, 3 * 128], BF, tag="wqkv")
    nc.sync.dma_start(out=wqkv[:, :], in_=io["wqkv"][:, :])

    dma_q2 = nc.scalar if ACT_Q_DMA else nc.sync
    qidx = const.tile([128, NJ], I32, tag="qidx")
    dma_q2.dma_start(out=qidx[:, :], in_=io["qidx"][:, :])

    # blob: sharp (0:64) | wzs_spT (64:68) | E_bc (68:84), all bf16
    blob = const.tile([128, 84], BF, tag="blob")
    nc.sync.dma_start(out=blob[:, :], in_=io["blob"][:, :])
    sharp = blob[:, 0:64]
    wzs = blob[:, 64:68]
    ebc = blob[:, 68:84]

    for b in range(1, NB):
        cet[b] = const.tile([EMB + 1, S], BF, tag=f"cet{b}", name=f"cet{b}")
        nc.sync.dma_start(out=cet[b][:, :], in_=io["ce"][b, :, :])

    # ---------------- small constants on idle engines --------------------
    ones_bf = const.tile([128, ATTN], BF, tag="ones_bf")
    nc.gpsimd.memset(ones_bf[:, :], 1.0)
    warm = const.tile([128, 128], BF, tag="warm")
    nc.vector.memset(warm[:, :], 0.5)

    # PE warm-up: dense matmuls at kernel start trip the HAM activity
    # window so the whole kernel runs at 2.4 GHz instead of 1.2.
    warm_ps = ps_misc.tile([128, 128], F32, tag="misc", name="warm_ps")
    for _ in range(WARMUP):
        nc.tensor.matmul(warm_ps[0:32, :], warm[:, 0:32], warm[:, :],
                         start=True, stop=True)

    # ---------------- gathers (gpsimd queue; independent of attention) ---
    # g_all free layout: j (16) x e (64) x c (4); one indirect DMA per j.
    g_all = big.tile([128, NJ * C * EMB], BF, tag="g_all")
    for j in range(NJ):
        nc.gpsimd.indirect_dma_start(
            out=g_all[:, C * EMB * j : C * EMB * (j + 1)],
            out_offset=None,
            in_=io["w3T"][:, :],
            in_offset=IndirectOffsetOnAxis(ap=qidx[:, j : j + 1], axis=0),
        )

    fw = big.tile([128, NJ * C], BF, tag="fw")
    out_all = big.tile([128, NJ * EMB], BF, tag="out_all")

    # ---------------- per-batch stages ------------------------------------
    qs_l, ks_l, v_l = {}, {}, {}

    def stage_qk(b):
        ceT = cet[b]
        qs_ps = ps_misc.tile([128, S], F32, tag="misc", name="qs_ps")
        for h in range(HEADS):
            nc.tensor.matmul(
                qs_ps[32 * h : 32 * (h + 1), :],
                wqkv[:, 32 * h : 32 * (h + 1)],
                ceT[:, :],
                start=True, stop=True,
                tile_position=(0, 32 * h),
            )
        qs = sb.tile([128, S], BF, tag="qs", name="qs")
        nc.vector.tensor_copy(qs[:, :], qs_ps[:, :])
        qs_l[b] = qs
        ks_ps = ps_misc.tile([128, S], F32, tag="misc", name="ks_ps")
        for h in range(HEADS):
            nc.tensor.matmul(
                ks_ps[32 * h : 32 * (h + 1), :],
                wqkv[:, 128 + 32 * h : 128 + 32 * (h + 1)],
                ceT[:, :],
                start=True, stop=True,
                tile_position=(0, 32 * h),
            )
        ks = sb.tile([128, S], BF, tag="ks", name="ks")
        # b=0 is ramp-critical: evac ks on the (still idle) ACT engine so the
        # qs/ks casts run in parallel; copy shares the Exp activation table.
        if b == 0:
            nc.scalar.copy(ks[:, :], ks_ps[:, :])
        else:
            nc.vector.tensor_copy(ks[:, :], ks_ps[:, :])
        ks_l[b] = ks

    def stage_v(b):
        ceT = cet[b]
        v_ps = ps_misc.tile([128, S], F32, tag="misc", name="v_ps")
        for cc in range(4):
            nc.tensor.matmul(
                v_ps[:, 128 * cc : 128 * (cc + 1)],
                ceT[:, 128 * cc : 128 * (cc + 1)],
                wqkv[:, 256:384],
                start=True, stop=True,
            )
        v_sp = sb.tile([128, S], BF, tag="v_sp", name="v_sp")
        nc.vector.tensor_copy(v_sp[:, :], v_ps[:, :])
        v_l[b] = v_sp

    # ---- scores/exp/AV software pipeline --------------------------------
    def qk_wave(b, cc):
        qs, ks = qs_l[b], ks_l[b]
        ets = []
        tiles = []
        for hh in range(2):  # head halves
            sc_ps = ps_scores.tile([128, 2 * S], F32, tag="scores")
            for hi in range(2):
                h = 2 * hh + hi
                if MERGED_QK:
                    nc.tensor.matmul(
                        sc_ps[:, S * hi : S * (hi + 1)],
                        ks[32 * h : 32 * h + HD, 128 * cc : 128 * (cc + 1)],
                        qs[32 * h : 32 * h + HD, :],
                        start=True,
                        stop=True,
                        tile_position=(32 * h, 0),
                    )
                else:
                    for jj in range(4):
                        nc.tensor.matmul(
                            sc_ps[32 * jj : 32 * (jj + 1), S * hi : S * (hi + 1)],
                            ks[32 * h : 32 * h + HD,
                               128 * cc + 32 * jj : 128 * cc + 32 * (jj + 1)],
                            qs[32 * h : 32 * h + HD, :],
                            start=True,
                            stop=True,
                            tile_position=(32 * h, 32 * jj),
                        )
            tiles.append(sc_ps)
        for sc_ps in tiles:
            et = expp.tile([128, 2 * S], BF, tag="expT")
            nc.scalar.activation(et[:, :], sc_ps[:, :], ACTF.Exp, scale=SCALE)
            ets.append(et)
        return ets

    def av_wave(b, cc, avt_ps, sums_ps, ets):
        for h in range(HEADS):
            mv = ets[h // 2][:, S * (h % 2) : S * (h % 2 + 1)]
            nc.tensor.matmul(
                avt_ps[32 * h : 32 * (h + 1), :],
                v_l[b][:, 128 * cc + 32 * h : 128 * cc + 32 * (h + 1)],
                mv,
                start=(cc == 0),
                stop=(cc == 3),
                tile_position=(0, 32 * h),
                skip_group_check=True,
            )
        for h in range(HEADS):
            mv = ets[h // 2][:, S * (h % 2) : S * (h % 2 + 1)]
            nc.tensor.matmul(
                sums_ps[32 * h : 32 * (h + 1), :],
                ones_bf[:, :],
                mv,
                start=(cc == 0),
                stop=(cc == 3),
                tile_position=(0, 32 * h),
                skip_group_check=True,
            )

    def post_batch(b, avt_ps, sums_ps):
        rec = sb.tile([128, S], F32, tag="rec")
        nc.vector.reciprocal_approx_fast(rec[:, :], sums_ps[:, :])
        # normT in halves so the z matmuls for cc 0-1 start while the
        # second half normalizes (suppression chain is the serial tail).
        normT = sb.tile([128, S], BF, tag="normT")
        z_ps = ps_misc.tile([128, 4 * C], F32, tag="misc", name="z_ps")
        for half in range(2):
            sl = slice(256 * half, 256 * (half + 1))
            nc.vector.tensor_tensor(
                normT[:, sl], avt_ps[:, sl], rec[:, sl], op=ALU.mult
            )
            for cc in (2 * half, 2 * half + 1):
                nc.tensor.matmul(
                    z_ps[:, C * cc : C * (cc + 1)],
                    normT[:, 128 * cc : 128 * (cc + 1)],
                    wzs[:, :],
                    start=True,
                    stop=True,
                )
        # u = 2 - sigmoid(z+bz) = (2+t)/(1+t) = 1 + 1/(1+t), t = e^z * e^bz
        tb = sb.tile([128, 4 * C], F32, tag="tb")
        nc.scalar.activation(tb[:, :], z_ps[:, :], ACTF.Exp)
        if not skip_ebc:
            tbe = sb.tile([128, 4 * C], F32, tag="tbe")
            nc.vector.tensor_tensor(tbe[:, :], tb[:, :], ebc[:, :], op=ALU.mult)
            tb = tbe
        ab = sb.tile([128, 4 * C], F32, tag="ab")
        nc.vector.tensor_scalar_add(ab[:, :], tb[:, :], 1.0)
        rb = sb.tile([128, 4 * C], F32, tag="rb")
        nc.vector.reciprocal_approx_fast(rb[:, :], ab[:, :])
        # fw = u * sharp = (1 + rb) * sharp  (0.5/head-mean folded in sharp)
        nc.vector.scalar_tensor_tensor(
            fw[:, 16 * b : 16 * (b + 1)],
            rb[:, :],
            1.0,
            sharp[:, 16 * b : 16 * (b + 1)],
            op0=ALU.add,
            op1=ALU.mult,
        )

        # final gather-contract: out[j,e] = sum_c g[j,e,c] * fw[j,c]
        # split across DVE and Pool for the last batch to shorten the tail.
        def gview(j0, j1):
            gsl = g_all[:, C * EMB * (NB * b + j0) : C * EMB * (NB * b + j1)]
            gv = gsl.rearrange("p (j e c) -> p j e c", e=EMB, c=C)
            fv = fw[:, 16 * b + C * j0 : 16 * b + C * j1].rearrange(
                "p (j c) -> p j c", c=C
            )[:, :, None, :].to_broadcast([128, j1 - j0, EMB, C])
            return gv, fv

        def reduce_half(j0, j1):
            gv, _ = gview(j0, j1)
            osl = out_all[:, EMB * (NB * b + j0) : EMB * (NB * b + j1)]
            nc.vector.tensor_reduce(
                osl.rearrange("p (j e) -> p j e", e=EMB),
                gv,
                axis=mybir.AxisListType.X,
                op=ALU.add,
            )

        def store(j0, j1):
            nc.sync.dma_start(
                out=io["out"][:, NB * b + j0 : NB * b + j1, :],
                in_=out_all[
                    :, EMB * (NB * b + j0) : EMB * (NB * b + j1)
                ].rearrange("p (j e) -> p j e", e=EMB),
            )

        if b == NB - 1:
            gv0, fv0 = gview(0, 2)
            nc.vector.tensor_tensor(gv0, gv0, fv0, op=ALU.mult)
            gv1, fv1 = gview(2, 4)
            nc.gpsimd.tensor_tensor(gv1, gv1, fv1, op=ALU.mult)
            reduce_half(0, 2)
            store(0, 2)
            reduce_half(2, 4)
            store(2, 4)
        else:
            gv0, fv0 = gview(0, 4)
            nc.vector.tensor_tensor(gv0, gv0, fv0, op=ALU.mult)
            reduce_half(0, 4)
            store(0, 4)

    ets_l = {b: [None] * 4 for b in range(NB)}
    av_tiles = {}

    def get_av(b):
        if b not in av_tiles:
            av_tiles[b] = (
                ps_av.tile([128, S], F32, tag="avt", name="avt_ps"),
                ps_sum.tile([128, S], F32, tag="sums", name="sums_ps"),
            )
        return av_tiles[b]

    stage_qk(0)
    prev = None  # (b, cc) whose AV wave is pending, 1 step behind
    for b in range(NB):
        for cc in range(4):
            ets_l[b][cc] = qk_wave(b, cc)
            if b == 0 and cc == 0:
                stage_v(0)
            if b + 1 < NB:
                # staged mid-batch: the boundary waves already carry the
                # previous batch's AV drain + post work on the PE queue.
                if cc == 1:
                    stage_qk(b + 1)
                elif cc == 2:
                    stage_v(b + 1)
            if prev is not None:
                pb, pcc = prev
                av_wave(pb, pcc, *get_av(pb), ets_l[pb][pcc])
                if pcc == 3:
                    post_batch(pb, *av_tiles[pb])
            prev = (b, cc)
    av_wave(NB - 1, 3, *get_av(NB - 1), ets_l[NB - 1][3])
    post_batch(NB - 1, *av_tiles[NB - 1])


# ======================= host side =======================

def _prep_weights(inp):
    """Pure host-side folding of the (small, replicated) parameters."""
    f32 = np.float32

    def bf(x):
        return np.ascontiguousarray(np.asarray(x, f32).astype(BF16))

    W_ctx = np.asarray(inp["W_ctx"], f32)
    W_in = np.asarray(inp["W_in"], f32)
    W_out = np.asarray(inp["W_out"], f32)
    W_sup = np.asarray(inp["W_sup"], f32)
    W_emb = np.asarray(inp["W_emb"], f32)
    b_ctx = np.asarray(inp["b_ctx"], f32)
    b_in = np.asarray(inp["b_in"], f32)
    b_out = np.asarray(inp["b_out"], f32)
    b_sup = np.asarray(inp["b_sup"], f32)
    temp = np.asarray(inp["temperature"], np.float64)

    w = {}

    # q/k/v projections folded with the ctx projection; bias via ones row.
    def spread(M, c):
        # M (32, 64) rows 8h+d, c (32,) -> (65, 128) cols 32h+d
        out = np.zeros((EMB + 1, 128), f32)
        for h in range(HEADS):
            out[0:EMB, 32 * h : 32 * h + HD] = M[HD * h : HD * (h + 1), :].T
            out[EMB, 32 * h : 32 * h + HD] = c[HD * h : HD * (h + 1)]
        return out

    parts = []
    for i in range(3):
        Wp = W_in[ATTN * i : ATTN * (i + 1)]
        bp = b_in[ATTN * i : ATTN * (i + 1)]
        parts.append(spread(Wp @ W_ctx, Wp @ b_ctx + bp))
    w["wqkv"] = bf(np.concatenate(parts, axis=1))  # (65, 384)

    # suppression: z = o @ (W_sup W_out)^T + (W_sup b_out + b_sup)
    Wz = W_sup @ W_out            # (4, 32)
    bz = W_sup @ b_out + b_sup    # (4,)
    wzs = np.zeros((128, C), f32)
    for h in range(HEADS):
        wzs[32 * h : 32 * h + HD, :] = Wz[:, HD * h : HD * (h + 1)].T
    ebc = np.broadcast_to(np.tile(np.exp(bz).astype(f32), 4)[None, :], (128, 16))

    # ordinal-softmax table: tbl[r, c] = 0.5 * mean_h softmax_c(bw(r,.)/T_h)
    kk = np.arange(C, dtype=np.float64)
    tbl = np.zeros((C, C), np.float64)
    for r in range(C):
        bw = np.clip(1.0 - np.abs(kk - r) / (C - 1), 0.0, None)
        sh = np.exp(bw[None, :] / temp[:, None])
        sh /= sh.sum(axis=1, keepdims=True)
        tbl[r] = 0.5 * sh.mean(axis=0)
    w["_tbl"] = tbl.astype(f32)
    w["_wzs"] = wzs
    w["_ebc"] = np.ascontiguousarray(ebc, dtype=f32)

    # gather table: w3T[q, 4e+c] = W_emb[e, c*Q+q]
    w["w3T"] = bf(W_emb.reshape(EMB, C, Q).transpose(2, 0, 1).reshape(Q, EMB * C))
    return w


def _spec():
    return {
        "wqkv": ((EMB + 1, 3 * 128), BF),
        "blob": ((128, 84), BF),
        "qidx": ((128, NJ), I32),
        "w3T": ((Q, C * EMB), BF),
        "ce": ((NB, EMB + 1, S), BF),
    }


def build_bass(skip_ebc: bool = False):
    nc = bacc.Bacc("TRN2", target_bir_lowering=False, debug=False)
    io = {}
    for name, (shape, dt) in _spec().items():
        io[name] = nc.dram_tensor(name, list(shape), dt, kind="ExternalInput").ap()
    io["out"] = nc.dram_tensor("out", [128, NJ, EMB], BF, kind="ExternalOutput").ap()
    with tile.TileContext(nc) as tc:
        build_kernel(nc, tc, io, skip_ebc)
    nc.compile()
    return nc


def skip_ebc_for(inputs) -> bool:
    """True when W_sup@b_out + b_sup == 0, so e^bz == 1 can be elided."""
    W_sup = np.asarray(inputs["W_sup"], np.float64)
    b_out = np.asarray(inputs["b_out"], np.float64)
    b_sup = np.asarray(inputs["b_sup"], np.float64)
    return bool(np.all(W_sup @ b_out + b_sup == 0.0))


def make_in_maps(inputs):
    inp = dict(inputs)
    w = _prep_weights(inp)
    tbl, wzs, ebc = w.pop("_tbl"), w.pop("_wzs"), w.pop("_ebc")
    q_idx = np.asarray(inp["q_idx"]).astype(np.int32)
    r_data = np.asarray(inp["r_data"]).astype(np.int64)
    ce = np.asarray(inp["context_embedding"], np.float32)

    in_maps = []
    for k in range(NCORES):
        m = dict(w)
        qs = q_idx[NB * k : NB * (k + 1)]          # (4,512)
        rs = r_data[NB * k : NB * (k + 1)]
        # token-tile layout: [p, j] with j = 4*b + cc, s = 128*cc + p
        m["qidx"] = np.ascontiguousarray(
            qs.reshape(NB, 4, 128).transpose(2, 0, 1).reshape(128, NJ)
        )
        # sharp values per token: (128, 64) = (p, j*4+c)
        sharp = tbl[rs.reshape(NB, 4, 128)]        # (b, cc, p, c)
        sharp = sharp.transpose(2, 0, 1, 3).reshape(128, NJ * C)
        blob = np.zeros((128, 84), np.float32)
        blob[:, 0:64] = sharp
        blob[:, 64:68] = wzs
        blob[:, 68:84] = ebc
        m["blob"] = blob.astype(BF16)
        # ceT with ones row: (NB, 65, 512)
        cek = ce[NB * k : NB * (k + 1)]            # (4, 512, 64)
        cet = np.ones((NB, EMB + 1, S), np.float32)
        cet[:, 0:EMB, :] = cek.transpose(0, 2, 1)
        m["ce"] = cet.astype(BF16)
        in_maps.append(m)
    return in_maps


_NC_CACHE = {}


def postprocess(res, inputs) -> np.ndarray:
    b_emb = np.asarray(inputs["b_emb"], np.float32)
    outs = []
    for k in range(NCORES):
        o = np.asarray(res.results[k]["out"]).astype(np.float32)  # (128,16,64)
        o = o.reshape(128, NB, 4, EMB).transpose(1, 2, 0, 3).reshape(NB, S, EMB)
        outs.append(o)
    out = np.concatenate(outs, axis=0) + b_emb[None, None, :]
    return out.astype(np.float32)


def kernel(**inputs) -> np.ndarray:
    key = skip_ebc_for(inputs)
    if key not in _NC_CACHE:
        _NC_CACHE[key] = build_bass(skip_ebc=key)
    nc = _NC_CACHE[key]
    in_maps = make_in_maps(inputs)
    res = run_bass_kernel_spmd(nc, in_maps, core_ids=list(range(NCORES)))
    return postprocess(res, inputs)


# revision 28
# speedup vs baseline: 1.0176x; 1.0155x over previous
"""Trainium2 Bass kernel for nn_AttentionModulatedOrdinalEmbedding.

Contract: kernel(**inputs) takes the FULL (unsharded) inputs from
setup_inputs() and returns the FULL (B, S, EMB) float32 output.
Internally shards batch-parallel across 8 NeuronCores (4 batches/core),
runs one SPMD Bass kernel, and concatenates the per-core outputs.

Hardcoded problem shape: B=32, S=512, N_Q=1024, N_CATS=4, EMB=64,
ATTN=32, HEADS=4 (head_dim 8).

Device-side structure (per core, 4 batches of 512 tokens):
- All weight algebra is folded on the host: ctx projection into q/k/v
  (with biases via an appended ones row of ceT), W_out into W_sup, and
  the ordinal-softmax path into a per-token (128,64) `sharp` table.
- The exp over the (4 heads x 512 x 512) score matrix per batch is the
  ACT-engine bottleneck; everything else is scheduled around keeping
  that stream dense: PE runs one wave ahead, DVE/Pool do evacuations
  and the final gather-contract, DMAs are front-loaded on the SP queue.
"""

import os
import sys
from contextlib import ExitStack

import numpy as np

for _p in ("/opt/trn_rl_repo", "/root/.axon_site/_ro/trn_rl_repo"):
    if os.path.isdir(_p) and _p not in sys.path:
        sys.path.append(_p)

import ml_dtypes  # noqa: E402

import concourse.bass as bass  # noqa: E402
import concourse.tile as tile  # noqa: E402
from concourse import bacc, mybir  # noqa: E402
from concourse.bass import IndirectOffsetOnAxis  # noqa: E402
from concourse.bass_utils import run_bass_kernel_spmd  # noqa: E402

BF16 = ml_dtypes.bfloat16
F32 = mybir.dt.float32
BF = mybir.dt.bfloat16
I32 = mybir.dt.int32
ALU = mybir.AluOpType
ACTF = mybir.ActivationFunctionType

B, S, EMB, ATTN, HEADS, HD, C, Q = 32, 512, 64, 32, 4, 8, 4, 1024
NCORES = 8
NB = B // NCORES          # batches per core = 4
NJ = NB * (S // 128)      # token tiles per core = 16
SCALE = 1.0 / np.sqrt(HD)
WARMUP = 18               # PE warm matmuls to trip the HAM clock boost
MERGED_QK = os.environ.get("K_MERGED_QK", "1") == "1"
ACT_Q_DMA = os.environ.get("K_ACT_DMA", "1") == "1"


def build_kernel(nc: bacc.Bacc, tc: tile.TileContext, io: dict, skip_ebc: bool):
    ctx = ExitStack()
    with ctx, nc.allow_low_precision(reason="bf16 accum within tolerance"):
        _build(nc, tc, ctx, io, skip_ebc)


def _build(nc, tc, ctx, io, skip_ebc):
    const = ctx.enter_context(tc.tile_pool(name="const", bufs=1))
    sb = ctx.enter_context(tc.tile_pool(name="sb", bufs=2))
    expp = ctx.enter_context(tc.tile_pool(name="expp", bufs=32))
    big = ctx.enter_context(tc.tile_pool(name="big", bufs=1))
    ps_scores = ctx.enter_context(tc.tile_pool(name="ps_scores", bufs=2, space="PSUM"))
    ps_av = ctx.enter_context(tc.tile_pool(name="ps_av", bufs=1, space="PSUM"))
    ps_sum = ctx.enter_context(tc.tile_pool(name="ps_sum", bufs=1, space="PSUM"))
    ps_misc = ctx.enter_context(tc.tile_pool(name="ps_misc", bufs=2, space="PSUM"))

    # ---------------- DMA loads: ramp-critical ceT0+wqkv lead the SP
    # queue; qidx goes on the ACT queue (hwdge-capable, idle during ramp).
    cet = {}
    cet[0] = const.tile([EMB + 1, S], BF, tag="cet0", name="cet0")
    nc.sync.dma_start(out=cet[0][:, :], in_=io["ce"][0, :, :])

    wqkv = const.tile([EMB + 1, 3 * 128], BF, tag="wqkv")
    nc.sync.dma_start(out=wqkv[:, :], in_=io["wqkv"][:, :])

    dma_q2 = nc.scalar if ACT_Q_DMA else nc.sync
    qidx = const.tile([128, NJ], I32, tag="qidx")
    dma_q2.dma_start(out=qidx[:, :], in_=io["qidx"][:, :])

    # blob: sharp (0:64) | wzs_spT (64:68) | E_bc (68:84), all bf16
    blob = const.tile([128, 84], BF, tag="blob")
    nc.sync.dma_start(out=blob[:, :], in_=io["blob"][:, :])
    sharp = blob[:, 0:64]
    wzs = blob[:, 64:68]
    ebc = blob[:, 68:84]

    for b in range(1, NB):
        cet[b] = const.tile([EMB + 1, S], BF, tag=f"cet{b}", name=f"cet{b}")
        nc.sync.dma_start(out=cet[b][:, :], in_=io["ce"][b, :, :])

    # ---------------- small constants on idle engines --------------------
    ones_bf = const.tile([128, ATTN], BF, tag="ones_bf")
    nc.gpsimd.memset(ones_bf[:, :], 1.0)
    warm = const.tile([128, 128], BF, tag="warm")
    nc.vector.memset(warm[:, :], 0.5)

    # PE warm-up: dense matmuls at kernel start trip the HAM activity
    # window so the whole kernel runs at 2.4 GHz instead of 1.2.
    warm_ps = ps_misc.tile([128, 128], F32, tag="misc", name="warm_ps")
    for _ in range(WARMUP):
        nc.tensor.matmul(warm_ps[0:32, :], warm[:, 0:32], warm[:, :],
                         start=True, stop=True)

    # ---------------- gathers (gpsimd queue; independent of attention) ---
    # g_all free layout: j (16) x e (64) x c (4); one indirect DMA per j.
    g_all = big.tile([128, NJ * C * EMB], BF, tag="g_all")
    for j in range(NJ):
        nc.gpsimd.indirect_dma_start(
            out=g_all[:, C * EMB * j : C * EMB * (j + 1)],
            out_offset=None,
            in_=io["w3T"][:, :],
            in_offset=IndirectOffsetOnAxis(ap=qidx[:, j : j + 1], axis=0),
        )

    fw = big.tile([128, NJ * C], BF, tag="fw")
    out_all = big.tile([128, NJ * EMB], BF, tag="out_all")

    # ---------------- per-batch stages ------------------------------------
    qs_l, ks_l, v_l = {}, {}, {}

    def stage_qk(b):
        ceT = cet[b]
        qs_ps = ps_misc.tile([128, S], F32, tag="misc", name="qs_ps")
        for h in range(HEADS):
            nc.tensor.matmul(
                qs_ps[32 * h : 32 * (h + 1), :],
                wqkv[:, 32 * h : 32 * (h + 1)],
                ceT[:, :],
                start=True, stop=True,
                tile_position=(0, 32 * h),
            )
        qs = sb.tile([128, S], BF, tag="qs", name="qs")
        nc.vector.tensor_copy(qs[:, :], qs_ps[:, :])
        qs_l[b] = qs
        ks_ps = ps_misc.tile([128, S], F32, tag="misc", name="ks_ps")
        for h in range(HEADS):
            nc.tensor.matmul(
                ks_ps[32 * h : 32 * (h + 1), :],
                wqkv[:, 128 + 32 * h : 128 + 32 * (h + 1)],
                ceT[:, :],
                start=True, stop=True,
                tile_position=(0, 32 * h),
            )
        ks = sb.tile([128, S], BF, tag="ks", name="ks")
        # b=0 is ramp-critical: evac ks on the (still idle) ACT engine so the
        # qs/ks casts run in parallel; copy shares the Exp activation table.
        if b == 0:
            nc.scalar.copy(ks[:, :], ks_ps[:, :])
        else:
            nc.vector.tensor_copy(ks[:, :], ks_ps[:, :])
        ks_l[b] = ks

    def stage_v(b):
        ceT = cet[b]
        v_ps = ps_misc.tile([128, S], F32, tag="misc", name="v_ps")
        for cc in range(4):
            nc.tensor.matmul(
                v_ps[:, 128 * cc : 128 * (cc + 1)],
                ceT[:, 128 * cc : 128 * (cc + 1)],
                wqkv[:, 256:384],
                start=True, stop=True,
            )
        v_sp = sb.tile([128, S], BF, tag="v_sp", name="v_sp")
        nc.vector.tensor_copy(v_sp[:, :], v_ps[:, :])
        v_l[b] = v_sp

    # ---- scores/exp/AV software pipeline --------------------------------
    def qk_wave(b, cc):
        qs, ks = qs_l[b], ks_l[b]
        ets = []
        tiles = []
        for hh in range(2):  # head halves
            sc_ps = ps_scores.tile([128, 2 * S], F32, tag="scores")
            for hi in range(2):
                h = 2 * hh + hi
                if MERGED_QK:
                    nc.tensor.matmul(
                        sc_ps[:, S * hi : S * (hi + 1)],
                        ks[32 * h : 32 * h + HD, 128 * cc : 128 * (cc + 1)],
                        qs[32 * h : 32 * h + HD, :],
                        start=True,
                        stop=True,
                        tile_position=(32 * h, 0),
                    )
                else:
                    for jj in range(4):
                        nc.tensor.matmul(
                            sc_ps[32 * jj : 32 * (jj + 1), S * hi : S * (hi + 1)],
                            ks[32 * h : 32 * h + HD,
                               128 * cc + 32 * jj : 128 * cc + 32 * (jj + 1)],
                            qs[32 * h : 32 * h + HD, :],
                            start=True,
                            stop=True,
                            tile_position=(32 * h, 32 * jj),
                        )
            tiles.append(sc_ps)
        for sc_ps in tiles:
            et = expp.tile([128, 2 * S], BF, tag="expT")
            nc.scalar.activation(et[:, :], sc_ps[:, :], ACTF.Exp, scale=SCALE)
            ets.append(et)
        return ets

    def av_wave(b, cc, avt_ps, sums_ps, ets):
        for h in range(HEADS):
            mv = ets[h // 2][:, S * (h % 2) : S * (h % 2 + 1)]
            nc.tensor.matmul(
                avt_ps[32 * h : 32 * (h + 1), :],
                v_l[b][:, 128 * cc + 32 * h : 128 * cc + 32 * (h + 1)],
                mv,
                start=(cc == 0),
                stop=(cc == 3),
                tile_position=(0, 32 * h),
                skip_group_check=True,
            )
        for h in range(HEADS):
            mv = ets[h // 2][:, S * (h % 2) : S * (h % 2 + 1)]
            nc.tensor.matmul(
                sums_ps[32 * h : 32 * (h + 1), :],
                ones_bf[:, :],
                mv,
                start=(cc == 0),
                stop=(cc == 3),
                tile_position=(0, 32 * h),
                skip_group_check=True,
            )

    def post_batch(b, avt_ps, sums_ps):
        rec = sb.tile([128, S], F32, tag="rec")
        nc.vector.reciprocal_approx_fast(rec[:, :], sums_ps[:, :])
        # normT in halves so the z matmuls for cc 0-1 start while the
        # second half normalizes (suppression chain is the serial tail).
        normT = sb.tile([128, S], BF, tag="normT")
        z_ps = ps_misc.tile([128, 4 * C], F32, tag="misc", name="z_ps")
        for half in range(2):
            sl = slice(256 * half, 256 * (half + 1))
            nc.vector.tensor_tensor(
                normT[:, sl], avt_ps[:, sl], rec[:, sl], op=ALU.mult
            )
            for cc in (2 * half, 2 * half + 1):
                nc.tensor.matmul(
                    z_ps[:, C * cc : C * (cc + 1)],
                    normT[:, 128 * cc : 128 * (cc + 1)],
                    wzs[:, :],
                    start=True,
                    stop=True,
                )
        # u = 2 - sigmoid(z+bz) = (2+t)/(1+t) = 1 + 1/(1+t), t = e^z * e^bz
        tb = sb.tile([128, 4 * C], F32, tag="tb")
        nc.scalar.activation(tb[:, :], z_ps[:, :], ACTF.Exp)
        if not skip_ebc:
            tbe = sb.tile([128, 4 * C], F32, tag="tbe")
            nc.vector.tensor_tensor(tbe[:, :], tb[:, :], ebc[:, :], op=ALU.mult)
            tb = tbe
        ab = sb.tile([128, 4 * C], F32, tag="ab")
        nc.vector.tensor_scalar_add(ab[:, :], tb[:, :], 1.0)
        rb = sb.tile([128, 4 * C], F32, tag="rb")
        nc.vector.reciprocal_approx_fast(rb[:, :], ab[:, :])
        # fw = u * sharp = (1 + rb) * sharp  (0.5/head-mean folded in sharp)
        nc.vector.scalar_tensor_tensor(
            fw[:, 16 * b : 16 * (b + 1)],
            rb[:, :],
            1.0,
            sharp[:, 16 * b : 16 * (b + 1)],
            op0=ALU.add,
            op1=ALU.mult,
        )

        # final gather-contract: out[j,e] = sum_c g[j,e,c] * fw[j,c]
        # split across DVE and Pool for the last batch to shorten the tail.
        def gview(j0, j1):
            gsl = g_all[:, C * EMB * (NB * b + j0) : C * EMB * (NB * b + j1)]
            gv = gsl.rearrange("p (j e c) -> p j e c", e=EMB, c=C)
            fv = fw[:, 16 * b + C * j0 : 16 * b + C * j1].rearrange(
                "p (j c) -> p j c", c=C
            )[:, :, None, :].to_broadcast([128, j1 - j0, EMB, C])
            return gv, fv

        def reduce_half(j0, j1):
            gv, _ = gview(j0, j1)
            osl = out_all[:, EMB * (NB * b + j0) : EMB * (NB * b + j1)]
            nc.vector.tensor_reduce(
                osl.rearrange("p (j e) -> p j e", e=EMB),
                gv,
                axis=mybir.AxisListType.X,
                op=ALU.add,
            )

        def store(j0, j1):
            nc.sync.dma_start(
                out=io["out"][:, NB * b + j0 : NB * b + j1, :],
                in_=out_all[
                    :, EMB * (NB * b + j0) : EMB * (NB * b + j1)
                ].rearrange("p (j e) -> p j e", e=EMB),
            )

        if b == NB - 1:
            gv0, fv0 = gview(0, 2)
            nc.vector.tensor_tensor(gv0, gv0, fv0, op=ALU.mult)
            gv1, fv1 = gview(2, 4)
            nc.gpsimd.tensor_tensor(gv1, gv1, fv1, op=ALU.mult)
            reduce_half(0, 2)
            store(0, 2)
            reduce_half(2, 4)
            store(2, 4)
        else:
            gv0, fv0 = gview(0, 4)
            nc.vector.tensor_tensor(gv0, gv0, fv0, op=ALU.mult)
            reduce_half(0, 4)
            store(0, 4)

    ets_l = {b: [None] * 4 for b in range(NB)}
    av_tiles = {}

    def get_av(b):
        if b not in av_tiles:
            av_tiles[b] = (
                ps_av.tile([128, S], F32, tag="avt", name="avt_ps"),
                ps_sum.tile([128, S], F32, tag="sums", name="sums_ps"),
            )
        return av_tiles[b]

    stage_qk(0)
    prev = None  # (b, cc) whose AV wave is pending, 1 step behind
    for b in range(NB):
        for cc in range(4):
            ets_l[b][cc] = qk_wave(b, cc)
            if b == 0 and cc == 0:
                stage_v(0)
            if b + 1 < NB:
                if cc == 0:
                    stage_qk(b + 1)
                elif cc == 1:
                    stage_v(b + 1)
            if prev is not None:
                pb, pcc = prev
                av_wave(pb, pcc, *get_av(pb), ets_l[pb][pcc])
                if pcc == 3:
                    post_batch(pb, *av_tiles[pb])
            prev = (b, cc)
    av_wave(NB - 1, 3, *get_av(NB - 1), ets_l[NB - 1][3])
    post_batch(NB - 1, *av_tiles[NB - 1])


# ======================= host side =======================

def _prep_weights(inp):
    """Pure host-side folding of the (small, replicated) parameters."""
    f32 = np.float32

    def bf(x):
        return np.ascontiguousarray(np.asarray(x, f32).astype(BF16))

    W_ctx = np.asarray(inp["W_ctx"], f32)
    W_in = np.asarray(inp["W_in"], f32)
    W_out = np.asarray(inp["W_out"], f32)
    W_sup = np.asarray(inp["W_sup"], f32)
    W_emb = np.asarray(inp["W_emb"], f32)
    b_ctx = np.asarray(inp["b_ctx"], f32)
    b_in = np.asarray(inp["b_in"], f32)
    b_out = np.asarray(inp["b_out"], f32)
    b_sup = np.asarray(inp["b_sup"], f32)
    temp = np.asarray(inp["temperature"], np.float64)

    w = {}

    # q/k/v projections folded with the ctx projection; bias via ones row.
    def spread(M, c):
        # M (32, 64) rows 8h+d, c (32,) -> (65, 128) cols 32h+d
        out = np.zeros((EMB + 1, 128), f32)
        for h in range(HEADS):
            out[0:EMB, 32 * h : 32 * h + HD] = M[HD * h : HD * (h + 1), :].T
            out[EMB, 32 * h : 32 * h + HD] = c[HD * h : HD * (h + 1)]
        return out

    parts = []
    for i in range(3):
        Wp = W_in[ATTN * i : ATTN * (i + 1)]
        bp = b_in[ATTN * i : ATTN * (i + 1)]
        parts.append(spread(Wp @ W_ctx, Wp @ b_ctx + bp))
    w["wqkv"] = bf(np.concatenate(parts, axis=1))  # (65, 384)

    # suppression: z = o @ (W_sup W_out)^T + (W_sup b_out + b_sup)
    Wz = W_sup @ W_out            # (4, 32)
    bz = W_sup @ b_out + b_sup    # (4,)
    wzs = np.zeros((128, C), f32)
    for h in range(HEADS):
        wzs[32 * h : 32 * h + HD, :] = Wz[:, HD * h : HD * (h + 1)].T
    ebc = np.broadcast_to(np.tile(np.exp(bz).astype(f32), 4)[None, :], (128, 16))

    # ordinal-softmax table: tbl[r, c] = 0.5 * mean_h softmax_c(bw(r,.)/T_h)
    kk = np.arange(C, dtype=np.float64)
    tbl = np.zeros((C, C), np.float64)
    for r in range(C):
        bw = np.clip(1.0 - np.abs(kk - r) / (C - 1), 0.0, None)
        sh = np.exp(bw[None, :] / temp[:, None])
        sh /= sh.sum(axis=1, keepdims=True)
        tbl[r] = 0.5 * sh.mean(axis=0)
    w["_tbl"] = tbl.astype(f32)
    w["_wzs"] = wzs
    w["_ebc"] = np.ascontiguousarray(ebc, dtype=f32)

    # gather table: w3T[q, 4e+c] = W_emb[e, c*Q+q]
    w["w3T"] = bf(W_emb.reshape(EMB, C, Q).transpose(2, 0, 1).reshape(Q, EMB * C))
    return w


def _spec():
    return {
        "wqkv": ((EMB + 1, 3 * 128), BF),
        "blob": ((128, 84), BF),
        "qidx": ((128, NJ), I32),
        "w3T": ((Q, C * EMB), BF),
        "ce": ((NB, EMB + 1, S), BF),
    }


def build_bass(skip_ebc: bool = False):
    nc = bacc.Bacc("TRN2", target_bir_lowering=False, debug=False)
    io = {}
    for name, (shape, dt) in _spec().items():
        io[name] = nc.dram_tensor(name, list(shape), dt, kind="ExternalInput").ap()
    io["out"] = nc.dram_tensor("out", [128, NJ, EMB], BF, kind="ExternalOutput").ap()
    with tile.TileContext(nc) as tc:
        build_kernel(nc, tc, io, skip_ebc)
    nc.compile()
    return nc


def skip_ebc_for(inputs) -> bool:
    """True when W_sup@b_out + b_sup == 0, so e^bz == 1 can be elided."""
    W_sup = np.asarray(inputs["W_sup"], np.float64)
    b_out = np.asarray(inputs["b_out"], np.float64)
    b_sup = np.asarray(inputs["b_sup"], np.float64)
    return bool(np.all(W_sup @ b_out + b_sup == 0.0))


def make_in_maps(inputs):
    inp = dict(inputs)
    w = _prep_weights(inp)
    tbl, wzs, ebc = w.pop("_tbl"), w.pop("_wzs"), w.pop("_ebc")
    q_idx = np.asarray(inp["q_idx"]).astype(np.int32)
    r_data = np.asarray(inp["r_data"]).astype(np.int64)
    ce = np.asarray(inp["context_embedding"], np.float32)

    in_maps = []
    for k in range(NCORES):
        m = dict(w)
        qs = q_idx[NB * k : NB * (k + 1)]          # (4,512)
        rs = r_data[NB * k : NB * (k + 1)]
        # token-tile layout: [p, j] with j = 4*b + cc, s = 128*cc + p
        m["qidx"] = np.ascontiguousarray(
            qs.reshape(NB, 4, 128).transpose(2, 0, 1).reshape(128, NJ)
        )
        # sharp values per token: (128, 64) = (p, j*4+c)
        sharp = tbl[rs.reshape(NB, 4, 128)]        # (b, cc, p, c)
        sharp = sharp.transpose(2, 0, 1, 3).reshape(128, NJ * C)
        blob = np.zeros((128, 84), np.float32)
        blob[:, 0:64] = sharp
        blob[:, 64:68] = wzs
        blob[:, 68:84] = ebc
        m["blob"] = blob.astype(BF16)
        # ceT with ones row: (NB, 65, 512)
        cek = ce[NB * k : NB * (k + 1)]            # (4, 512, 64)
        cet = np.ones((NB, EMB + 1, S), np.float32)
        cet[:, 0:EMB, :] = cek.transpose(0, 2, 1)
        m["ce"] = cet.astype(BF16)
        in_maps.append(m)
    return in_maps


_NC_CACHE = {}


def postprocess(res, inputs) -> np.ndarray:
    b_emb = np.asarray(inputs["b_emb"], np.float32)
    outs = []
    for k in range(NCORES):
        o = np.asarray(res.results[k]["out"]).astype(np.float32)  # (128,16,64)
        o = o.reshape(128, NB, 4, EMB).transpose(1, 2, 0, 3).reshape(NB, S, EMB)
        outs.append(o)
    out = np.concatenate(outs, axis=0) + b_emb[None, None, :]
    return out.astype(np.float32)


def kernel(**inputs) -> np.ndarray:
    key = skip_ebc_for(inputs)
    if key not in _NC_CACHE:
        _NC_CACHE[key] = build_bass(skip_ebc=key)
    nc = _NC_CACHE[key]
    in_maps = make_in_maps(inputs)
    res = run_bass_kernel_spmd(nc, in_maps, core_ids=list(range(NCORES)))
    return postprocess(res, inputs)


# revision 29
# speedup vs baseline: 1.0529x; 1.0347x over previous
"""Trainium2 Bass kernel for nn_AttentionModulatedOrdinalEmbedding.

Contract: kernel(**inputs) takes the FULL (unsharded) inputs from
setup_inputs() and returns the FULL (B, S, EMB) float32 output.
Internally shards batch-parallel across 8 NeuronCores (4 batches/core),
runs one SPMD Bass kernel, and concatenates the per-core outputs.

Hardcoded problem shape: B=32, S=512, N_Q=1024, N_CATS=4, EMB=64,
ATTN=32, HEADS=4 (head_dim 8).

Device-side structure (per core, 4 batches of 512 tokens):
- All weight algebra is folded on the host: ctx projection into q/k/v
  (with biases via an appended ones row of ceT), W_out into W_sup, and
  the ordinal-softmax path into a per-token (128,64) `sharp` table.
- The exp over the (4 heads x 512 x 512) score matrix per batch is the
  ACT-engine bottleneck; everything else is scheduled around keeping
  that stream dense: PE runs one wave ahead, DVE/Pool do evacuations
  and the final gather-contract, DMAs are front-loaded on the SP queue.
"""

import os
import sys
from contextlib import ExitStack

import numpy as np

for _p in ("/opt/trn_rl_repo", "/root/.axon_site/_ro/trn_rl_repo"):
    if os.path.isdir(_p) and _p not in sys.path:
        sys.path.append(_p)

import ml_dtypes  # noqa: E402

import concourse.bass as bass  # noqa: E402
import concourse.tile as tile  # noqa: E402
from concourse import bacc, mybir  # noqa: E402
from concourse.bass import IndirectOffsetOnAxis  # noqa: E402
from concourse.bass_utils import run_bass_kernel_spmd  # noqa: E402

BF16 = ml_dtypes.bfloat16
F32 = mybir.dt.float32
BF = mybir.dt.bfloat16
I32 = mybir.dt.int32
ALU = mybir.AluOpType
ACTF = mybir.ActivationFunctionType

B, S, EMB, ATTN, HEADS, HD, C, Q = 32, 512, 64, 32, 4, 8, 4, 1024
NCORES = 8
NB = B // NCORES          # batches per core = 4
NJ = NB * (S // 128)      # token tiles per core = 16
SCALE = 1.0 / np.sqrt(HD)
WARMUP = 18               # PE warm matmuls to trip the HAM clock boost
MERGED_QK = os.environ.get("K_MERGED_QK", "1") == "1"
ACT_Q_DMA = os.environ.get("K_ACT_DMA", "1") == "1"


def build_kernel(nc: bacc.Bacc, tc: tile.TileContext, io: dict, skip_ebc: bool):
    ctx = ExitStack()
    with ctx, nc.allow_low_precision(reason="bf16 accum within tolerance"):
        _build(nc, tc, ctx, io, skip_ebc)


def _build(nc, tc, ctx, io, skip_ebc):
    const = ctx.enter_context(tc.tile_pool(name="const", bufs=1))
    sb = ctx.enter_context(tc.tile_pool(name="sb", bufs=2))
    expp = ctx.enter_context(tc.tile_pool(name="expp", bufs=32))
    big = ctx.enter_context(tc.tile_pool(name="big", bufs=1))
    ps_scores = ctx.enter_context(tc.tile_pool(name="ps_scores", bufs=2, space="PSUM"))
    ps_av = ctx.enter_context(tc.tile_pool(name="ps_av", bufs=1, space="PSUM"))
    ps_sum = ctx.enter_context(tc.tile_pool(name="ps_sum", bufs=1, space="PSUM"))
    ps_misc = ctx.enter_context(tc.tile_pool(name="ps_misc", bufs=2, space="PSUM"))

    # ---------------- DMA loads: ramp-critical ceT0+wqkv lead the SP
    # queue; qidx goes on the ACT queue (hwdge-capable, idle during ramp).
    cet = {}
    cet[0] = const.tile([EMB + 1, S], BF, tag="cet0", name="cet0")
    nc.sync.dma_start(out=cet[0][:, :], in_=io["ce"][0, :, :])

    wqkv = const.tile([EMB + 1, 3 * 128], BF, tag="wqkv")
    nc.sync.dma_start(out=wqkv[:, :], in_=io["wqkv"][:, :])

    dma_q2 = nc.scalar if ACT_Q_DMA else nc.sync
    qidx = const.tile([128, NJ], I32, tag="qidx")
    dma_q2.dma_start(out=qidx[:, :], in_=io["qidx"][:, :])

    # blob: sharp (0:64) | wzs_spT (64:68) | E_bc (68:84), all bf16
    blob = const.tile([128, 84], BF, tag="blob")
    nc.sync.dma_start(out=blob[:, :], in_=io["blob"][:, :])
    sharp = blob[:, 0:64]
    wzs = blob[:, 64:68]
    ebc = blob[:, 68:84]

    for b in range(1, NB):
        cet[b] = const.tile([EMB + 1, S], BF, tag=f"cet{b}", name=f"cet{b}")
        nc.sync.dma_start(out=cet[b][:, :], in_=io["ce"][b, :, :])

    # ---------------- small constants on idle engines --------------------
    ones_bf = const.tile([128, ATTN], BF, tag="ones_bf")
    nc.gpsimd.memset(ones_bf[:, :], 1.0)
    warm = const.tile([128, 128], BF, tag="warm")
    nc.vector.memset(warm[:, :], 0.5)

    # PE warm-up: dense matmuls at kernel start trip the HAM activity
    # window so the whole kernel runs at 2.4 GHz instead of 1.2.
    warm_ps = ps_misc.tile([128, 128], F32, tag="misc", name="warm_ps")
    for _ in range(WARMUP):
        nc.tensor.matmul(warm_ps[0:32, :], warm[:, 0:32], warm[:, :],
                         start=True, stop=True)

    # ---------------- gathers (gpsimd queue; independent of attention) ---
    # g_all free layout: j (16) x e (64) x c (4); one indirect DMA per j.
    g_all = big.tile([128, NJ * C * EMB], BF, tag="g_all")
    for j in range(NJ):
        nc.gpsimd.indirect_dma_start(
            out=g_all[:, C * EMB * j : C * EMB * (j + 1)],
            out_offset=None,
            in_=io["w3T"][:, :],
            in_offset=IndirectOffsetOnAxis(ap=qidx[:, j : j + 1], axis=0),
        )

    fw = big.tile([128, NJ * C], BF, tag="fw")
    out_all = big.tile([128, NJ * EMB], BF, tag="out_all")

    # ---------------- per-batch stages ------------------------------------
    qs_l, ks_l, v_l = {}, {}, {}

    def stage_qk(b):
        ceT = cet[b]
        qs_ps = ps_misc.tile([128, S], F32, tag="misc", name="qs_ps")
        for h in range(HEADS):
            nc.tensor.matmul(
                qs_ps[32 * h : 32 * (h + 1), :],
                wqkv[:, 32 * h : 32 * (h + 1)],
                ceT[:, :],
                start=True, stop=True,
                tile_position=(0, 32 * h),
            )
        qs = sb.tile([128, S], BF, tag="qs", name="qs")
        nc.vector.tensor_copy(qs[:, :], qs_ps[:, :])
        qs_l[b] = qs
        ks_ps = ps_misc.tile([128, S], F32, tag="misc", name="ks_ps")
        for h in range(HEADS):
            nc.tensor.matmul(
                ks_ps[32 * h : 32 * (h + 1), :],
                wqkv[:, 128 + 32 * h : 128 + 32 * (h + 1)],
                ceT[:, :],
                start=True, stop=True,
                tile_position=(0, 32 * h),
            )
        ks = sb.tile([128, S], BF, tag="ks", name="ks")
        # b=0 is ramp-critical: evac ks on the (still idle) ACT engine so the
        # qs/ks casts run in parallel; copy shares the Exp activation table.
        if b == 0:
            nc.scalar.copy(ks[:, :], ks_ps[:, :])
        else:
            nc.vector.tensor_copy(ks[:, :], ks_ps[:, :])
        ks_l[b] = ks

    def stage_v(b):
        ceT = cet[b]
        v_ps = ps_misc.tile([128, S], F32, tag="misc", name="v_ps")
        for cc in range(4):
            nc.tensor.matmul(
                v_ps[:, 128 * cc : 128 * (cc + 1)],
                ceT[:, 128 * cc : 128 * (cc + 1)],
                wqkv[:, 256:384],
                start=True, stop=True,
            )
        v_sp = sb.tile([128, S], BF, tag="v_sp", name="v_sp")
        nc.vector.tensor_copy(v_sp[:, :], v_ps[:, :])
        v_l[b] = v_sp

    # ---- scores/exp/AV software pipeline --------------------------------
    def qk_wave(b, cc):
        qs, ks = qs_l[b], ks_l[b]
        ets = []
        tiles = []
        for hh in range(2):  # head halves
            sc_ps = ps_scores.tile([128, 2 * S], F32, tag="scores")
            for hi in range(2):
                h = 2 * hh + hi
                if MERGED_QK:
                    nc.tensor.matmul(
                        sc_ps[:, S * hi : S * (hi + 1)],
                        ks[32 * h : 32 * h + HD, 128 * cc : 128 * (cc + 1)],
                        qs[32 * h : 32 * h + HD, :],
                        start=True,
                        stop=True,
                        tile_position=(32 * h, 0),
                    )
                else:
                    for jj in range(4):
                        nc.tensor.matmul(
                            sc_ps[32 * jj : 32 * (jj + 1), S * hi : S * (hi + 1)],
                            ks[32 * h : 32 * h + HD,
                               128 * cc + 32 * jj : 128 * cc + 32 * (jj + 1)],
                            qs[32 * h : 32 * h + HD, :],
                            start=True,
                            stop=True,
                            tile_position=(32 * h, 32 * jj),
                        )
            tiles.append(sc_ps)
        for sc_ps in tiles:
            et = expp.tile([128, 2 * S], BF, tag="expT")
            nc.scalar.activation(et[:, :], sc_ps[:, :], ACTF.Exp, scale=SCALE)
            ets.append(et)
        return ets

    def av_wave(b, cc, avt_ps, sums_ps, ets):
        for h in range(HEADS):
            mv = ets[h // 2][:, S * (h % 2) : S * (h % 2 + 1)]
            nc.tensor.matmul(
                avt_ps[32 * h : 32 * (h + 1), :],
                v_l[b][:, 128 * cc + 32 * h : 128 * cc + 32 * (h + 1)],
                mv,
                start=(cc == 0),
                stop=(cc == 3),
                tile_position=(0, 32 * h),
                skip_group_check=True,
            )
        for h in range(HEADS):
            mv = ets[h // 2][:, S * (h % 2) : S * (h % 2 + 1)]
            nc.tensor.matmul(
                sums_ps[32 * h : 32 * (h + 1), :],
                ones_bf[:, :],
                mv,
                start=(cc == 0),
                stop=(cc == 3),
                tile_position=(0, 32 * h),
                skip_group_check=True,
            )

    def post_batch(b, avt_ps, sums_ps):
        rec = sb.tile([128, S], F32, tag="rec")
        nc.vector.reciprocal_approx_fast(rec[:, :], sums_ps[:, :])
        normT = sb.tile([128, S], BF, tag="normT")
        nc.vector.tensor_tensor(normT[:, :], avt_ps[:, :], rec[:, :], op=ALU.mult)
        z_ps = ps_misc.tile([128, 4 * C], F32, tag="misc", name="z_ps")
        for cc in range(4):
            nc.tensor.matmul(
                z_ps[:, C * cc : C * (cc + 1)],
                normT[:, 128 * cc : 128 * (cc + 1)],
                wzs[:, :],
                start=True,
                stop=True,
            )
        # u = 2 - sigmoid(z+bz) = (2+t)/(1+t) = 1 + 1/(1+t), t = e^z * e^bz
        tb = sb.tile([128, 4 * C], F32, tag="tb")
        nc.scalar.activation(tb[:, :], z_ps[:, :], ACTF.Exp)
        if not skip_ebc:
            tbe = sb.tile([128, 4 * C], F32, tag="tbe")
            nc.vector.tensor_tensor(tbe[:, :], tb[:, :], ebc[:, :], op=ALU.mult)
            tb = tbe
        ab = sb.tile([128, 4 * C], F32, tag="ab")
        nc.vector.tensor_scalar_add(ab[:, :], tb[:, :], 1.0)
        rb = sb.tile([128, 4 * C], F32, tag="rb")
        nc.vector.reciprocal_approx_fast(rb[:, :], ab[:, :])
        # fw = u * sharp = (1 + rb) * sharp  (0.5/head-mean folded in sharp)
        nc.vector.scalar_tensor_tensor(
            fw[:, 16 * b : 16 * (b + 1)],
            rb[:, :],
            1.0,
            sharp[:, 16 * b : 16 * (b + 1)],
            op0=ALU.add,
            op1=ALU.mult,
        )

        # final gather-contract: out[j,e] = sum_c g[j,e,c] * fw[j,c]
        # split across DVE and Pool for the last batch to shorten the tail.
        def gview(j0, j1):
            gsl = g_all[:, C * EMB * (NB * b + j0) : C * EMB * (NB * b + j1)]
            gv = gsl.rearrange("p (j e c) -> p j e c", e=EMB, c=C)
            fv = fw[:, 16 * b + C * j0 : 16 * b + C * j1].rearrange(
                "p (j c) -> p j c", c=C
            )[:, :, None, :].to_broadcast([128, j1 - j0, EMB, C])
            return gv, fv

        def reduce_half(j0, j1):
            gv, _ = gview(j0, j1)
            osl = out_all[:, EMB * (NB * b + j0) : EMB * (NB * b + j1)]
            nc.vector.tensor_reduce(
                osl.rearrange("p (j e) -> p j e", e=EMB),
                gv,
                axis=mybir.AxisListType.X,
                op=ALU.add,
            )

        def store(j0, j1):
            nc.sync.dma_start(
                out=io["out"][:, NB * b + j0 : NB * b + j1, :],
                in_=out_all[
                    :, EMB * (NB * b + j0) : EMB * (NB * b + j1)
                ].rearrange("p (j e) -> p j e", e=EMB),
            )

        if b == NB - 1:
            gv0, fv0 = gview(0, 2)
            nc.vector.tensor_tensor(gv0, gv0, fv0, op=ALU.mult)
            gv1, fv1 = gview(2, 4)
            nc.gpsimd.tensor_tensor(gv1, gv1, fv1, op=ALU.mult)
            reduce_half(0, 2)
            store(0, 2)
            reduce_half(2, 4)
            store(2, 4)
        else:
            gv0, fv0 = gview(0, 4)
            nc.vector.tensor_tensor(gv0, gv0, fv0, op=ALU.mult)
            reduce_half(0, 4)
            store(0, 4)

    ets_l = {b: [None] * 4 for b in range(NB)}
    av_tiles = {}

    def get_av(b):
        if b not in av_tiles:
            av_tiles[b] = (
                ps_av.tile([128, S], F32, tag="avt", name="avt_ps"),
                ps_sum.tile([128, S], F32, tag="sums", name="sums_ps"),
            )
        return av_tiles[b]

    stage_qk(0)
    prev = None  # (b, cc) whose AV wave is pending, 1 step behind
    for b in range(NB):
        for cc in range(4):
            ets_l[b][cc] = qk_wave(b, cc)
            if b == 0 and cc == 0:
                stage_v(0)
            if b + 1 < NB:
                if cc == 0:
                    stage_qk(b + 1)
                elif cc == 1:
                    stage_v(b + 1)
            if prev is not None:
                pb, pcc = prev
                av_wave(pb, pcc, *get_av(pb), ets_l[pb][pcc])
                if pcc == 3:
                    post_batch(pb, *av_tiles[pb])
            prev = (b, cc)
    av_wave(NB - 1, 3, *get_av(NB - 1), ets_l[NB - 1][3])
    post_batch(NB - 1, *av_tiles[NB - 1])


# ======================= host side =======================

def _prep_weights(inp):
    """Pure host-side folding of the (small, replicated) parameters."""
    f32 = np.float32

    def bf(x):
        return np.ascontiguousarray(np.asarray(x, f32).astype(BF16))

    W_ctx = np.asarray(inp["W_ctx"], f32)
    W_in = np.asarray(inp["W_in"], f32)
    W_out = np.asarray(inp["W_out"], f32)
    W_sup = np.asarray(inp["W_sup"], f32)
    W_emb = np.asarray(inp["W_emb"], f32)
    b_ctx = np.asarray(inp["b_ctx"], f32)
    b_in = np.asarray(inp["b_in"], f32)
    b_out = np.asarray(inp["b_out"], f32)
    b_sup = np.asarray(inp["b_sup"], f32)
    temp = np.asarray(inp["temperature"], np.float64)

    w = {}

    # q/k/v projections folded with the ctx projection; bias via ones row.
    def spread(M, c):
        # M (32, 64) rows 8h+d, c (32,) -> (65, 128) cols 32h+d
        out = np.zeros((EMB + 1, 128), f32)
        for h in range(HEADS):
            out[0:EMB, 32 * h : 32 * h + HD] = M[HD * h : HD * (h + 1), :].T
            out[EMB, 32 * h : 32 * h + HD] = c[HD * h : HD * (h + 1)]
        return out

    parts = []
    for i in range(3):
        Wp = W_in[ATTN * i : ATTN * (i + 1)]
        bp = b_in[ATTN * i : ATTN * (i + 1)]
        parts.append(spread(Wp @ W_ctx, Wp @ b_ctx + bp))
    w["wqkv"] = bf(np.concatenate(parts, axis=1))  # (65, 384)

    # suppression: z = o @ (W_sup W_out)^T + (W_sup b_out + b_sup)
    Wz = W_sup @ W_out            # (4, 32)
    bz = W_sup @ b_out + b_sup    # (4,)
    wzs = np.zeros((128, C), f32)
    for h in range(HEADS):
        wzs[32 * h : 32 * h + HD, :] = Wz[:, HD * h : HD * (h + 1)].T
    ebc = np.broadcast_to(np.tile(np.exp(bz).astype(f32), 4)[None, :], (128, 16))

    # ordinal-softmax table: tbl[r, c] = 0.5 * mean_h softmax_c(bw(r,.)/T_h)
    kk = np.arange(C, dtype=np.float64)
    tbl = np.zeros((C, C), np.float64)
    for r in range(C):
        bw = np.clip(1.0 - np.abs(kk - r) / (C - 1), 0.0, None)
        sh = np.exp(bw[None, :] / temp[:, None])
        sh /= sh.sum(axis=1, keepdims=True)
        tbl[r] = 0.5 * sh.mean(axis=0)
    w["_tbl"] = tbl.astype(f32)
    w["_wzs"] = wzs
    w["_ebc"] = np.ascontiguousarray(ebc, dtype=f32)

    # gather table: w3T[q, 4e+c] = W_emb[e, c*Q+q]
    w["w3T"] = bf(W_emb.reshape(EMB, C, Q).transpose(2, 0, 1).reshape(Q, EMB * C))
    return w


def _spec():
    return {
        "wqkv": ((EMB + 1, 3 * 128), BF),
        "blob": ((128, 84), BF),
        "qidx": ((128, NJ), I32),
        "w3T": ((Q, C * EMB), BF),
        "ce": ((NB, EMB + 1, S), BF),
    }


def build_bass(skip_ebc: bool = False):
    nc = bacc.Bacc("TRN2", target_bir_lowering=False, debug=False)
    io = {}
    for name, (shape, dt) in _spec().items():
        io[name] = nc.dram_tensor(name, list(shape), dt, kind="ExternalInput").ap()
    io["out"] = nc.dram_tensor("out", [128, NJ, EMB], BF, kind="ExternalOutput").ap()
    with tile.TileContext(nc) as tc:
        build_kernel(nc, tc, io, skip_ebc)
    nc.compile()
    return nc


def skip_ebc_for(inputs) -> bool:
    """True when W_sup@b_out + b_sup == 0, so e^bz == 1 can be elided."""
    W_sup = np.asarray(inputs["W_sup"], np.float64)
    b_out = np.asarray(inputs["b_out"], np.float64)
    b_sup = np.asarray(inputs["b_sup"], np.float64)
    return bool(np.all(W_sup @ b_out + b_sup == 0.0))


def make_in_maps(inputs):
    inp = dict(inputs)
    w = _prep_weights(inp)
    tbl, wzs, ebc = w.pop("_tbl"), w.pop("_wzs"), w.pop("_ebc")
    q_idx = np.asarray(inp["q_idx"]).astype(np.int32)
    r_data = np.asarray(inp["r_data"]).astype(np.int64)
    ce = np.asarray(inp["context_embedding"], np.float32)

    in_maps = []
    for k in range(NCORES):
        m = dict(w)
        qs = q_idx[NB * k : NB * (k + 1)]          # (4,512)
        rs = r_data[NB * k : NB * (k + 1)]
        # token-tile layout: [p, j] with j = 4*b + cc, s = 128*cc + p
        m["qidx"] = np.ascontiguousarray(
            qs.reshape(NB, 4, 128).transpose(2, 0, 1).reshape(128, NJ)
        )
        # sharp values per token: (128, 64) = (p, j*4+c)
        sharp = tbl[rs.reshape(NB, 4, 128)]        # (b, cc, p, c)
        sharp = sharp.transpose(2, 0, 1, 3).reshape(128, NJ * C)
        blob = np.zeros((128, 84), np.float32)
        blob[:, 0:64] = sharp
        blob[:, 64:68] = wzs
        blob[:, 68:84] = ebc
        m["blob"] = blob.astype(BF16)
        # ceT with ones row: (NB, 65, 512)
        cek = ce[NB * k : NB * (k + 1)]            # (4, 512, 64)
        cet = np.ones((NB, EMB + 1, S), np.float32)
        cet[:, 0:EMB, :] = cek.transpose(0, 2, 1)
        m["ce"] = cet.astype(BF16)
        in_maps.append(m)
    return in_maps


_NC_CACHE = {}


def postprocess(res, inputs) -> np.ndarray:
    b_emb = np.asarray(inputs["b_emb"], np.float32)
    outs = []
    for k in range(NCORES):
        o = np.asarray(res.results[k]["out"]).astype(np.float32)  # (128,16,64)
        o = o.reshape(128, NB, 4, EMB).transpose(1, 2, 0, 3).reshape(NB, S, EMB)
        outs.append(o)
    out = np.concatenate(outs, axis=0) + b_emb[None, None, :]
    return out.astype(np.float32)


def kernel(**inputs) -> np.ndarray:
    key = skip_ebc_for(inputs)
    if key not in _NC_CACHE:
        _NC_CACHE[key] = build_bass(skip_ebc=key)
    nc = _NC_CACHE[key]
    in_maps = make_in_maps(inputs)
    res = run_bass_kernel_spmd(nc, in_maps, core_ids=list(range(NCORES)))
    return postprocess(res, inputs)


# revision 30
# speedup vs baseline: 1.0664x; 1.0129x over previous
"""Trainium2 Bass kernel for nn_AttentionModulatedOrdinalEmbedding.

Contract: kernel(**inputs) takes the FULL (unsharded) inputs from
setup_inputs() and returns the FULL (B, S, EMB) float32 output.
Internally shards batch-parallel across 8 NeuronCores (4 batches/core),
runs one SPMD Bass kernel, and concatenates the per-core outputs.

Hardcoded problem shape: B=32, S=512, N_Q=1024, N_CATS=4, EMB=64,
ATTN=32, HEADS=4 (head_dim 8).

Device-side structure (per core, 4 batches of 512 tokens):
- All weight algebra is folded on the host: ctx projection into q/k/v
  (with biases via an appended ones row of ceT), W_out into W_sup, and
  the ordinal-softmax path into a per-token (128,64) `sharp` table.
- The exp over the (4 heads x 512 x 512) score matrix per batch is the
  ACT-engine bottleneck; everything else is scheduled around keeping
  that stream dense: PE runs one wave ahead, DVE/Pool do evacuations
  and the final gather-contract, DMAs are front-loaded on the SP queue.
"""

import os
import sys
from contextlib import ExitStack

import numpy as np

for _p in ("/opt/trn_rl_repo", "/root/.axon_site/_ro/trn_rl_repo"):
    if os.path.isdir(_p) and _p not in sys.path:
        sys.path.append(_p)

import ml_dtypes  # noqa: E402

import concourse.bass as bass  # noqa: E402
import concourse.tile as tile  # noqa: E402
from concourse import bacc, mybir  # noqa: E402
from concourse.bass import IndirectOffsetOnAxis  # noqa: E402
from concourse.bass_utils import run_bass_kernel_spmd  # noqa: E402

BF16 = ml_dtypes.bfloat16
F32 = mybir.dt.float32
BF = mybir.dt.bfloat16
I32 = mybir.dt.int32
ALU = mybir.AluOpType
ACTF = mybir.ActivationFunctionType

B, S, EMB, ATTN, HEADS, HD, C, Q = 32, 512, 64, 32, 4, 8, 4, 1024
NCORES = 8
NB = B // NCORES          # batches per core = 4
NJ = NB * (S // 128)      # token tiles per core = 16
SCALE = 1.0 / np.sqrt(HD)
WARMUP = 18               # PE warm matmuls to trip the HAM clock boost
MERGED_QK = os.environ.get("K_MERGED_QK", "1") == "1"
ACT_Q_DMA = os.environ.get("K_ACT_DMA", "1") == "1"


def build_kernel(nc: bacc.Bacc, tc: tile.TileContext, io: dict, skip_ebc: bool):
    ctx = ExitStack()
    with ctx, nc.allow_low_precision(reason="bf16 accum within tolerance"):
        _build(nc, tc, ctx, io, skip_ebc)


def _build(nc, tc, ctx, io, skip_ebc):
    const = ctx.enter_context(tc.tile_pool(name="const", bufs=1))
    sb = ctx.enter_context(tc.tile_pool(name="sb", bufs=2))
    expp = ctx.enter_context(tc.tile_pool(name="expp", bufs=32))
    big = ctx.enter_context(tc.tile_pool(name="big", bufs=1))
    ps_scores = ctx.enter_context(tc.tile_pool(name="ps_scores", bufs=2, space="PSUM"))
    ps_av = ctx.enter_context(tc.tile_pool(name="ps_av", bufs=1, space="PSUM"))
    ps_sum = ctx.enter_context(tc.tile_pool(name="ps_sum", bufs=1, space="PSUM"))
    ps_misc = ctx.enter_context(tc.tile_pool(name="ps_misc", bufs=2, space="PSUM"))

    # ---------------- DMA loads: wqkv on SP, ceT0/qidx on the ACT queue
    # (both hwdge-capable) so the two ramp-critical loads transfer in
    # parallel; everything else trails on SP.
    wqkv = const.tile([EMB + 1, 3 * 128], BF, tag="wqkv")
    nc.sync.dma_start(out=wqkv[:, :], in_=io["wqkv"][:, :])

    dma_q2 = nc.scalar if ACT_Q_DMA else nc.sync
    cet = {}
    cet[0] = const.tile([EMB + 1, S], BF, tag="cet0", name="cet0")
    dma_q2.dma_start(out=cet[0][:, :], in_=io["ce"][0, :, :])

    qidx = const.tile([128, NJ], I32, tag="qidx")
    dma_q2.dma_start(out=qidx[:, :], in_=io["qidx"][:, :])

    # blob: sharp (0:64) | wzs_spT (64:68) | E_bc (68:84), all bf16
    blob = const.tile([128, 84], BF, tag="blob")
    nc.sync.dma_start(out=blob[:, :], in_=io["blob"][:, :])
    sharp = blob[:, 0:64]
    wzs = blob[:, 64:68]
    ebc = blob[:, 68:84]

    for b in range(1, NB):
        cet[b] = const.tile([EMB + 1, S], BF, tag=f"cet{b}", name=f"cet{b}")
        nc.sync.dma_start(out=cet[b][:, :], in_=io["ce"][b, :, :])

    # ---------------- small constants on idle engines --------------------
    ones_bf = const.tile([128, ATTN], BF, tag="ones_bf")
    nc.gpsimd.memset(ones_bf[:, :], 1.0)
    warm = const.tile([128, 128], BF, tag="warm")
    nc.vector.memset(warm[:, :], 0.5)

    # PE warm-up: dense matmuls at kernel start trip the HAM activity
    # window so the whole kernel runs at 2.4 GHz instead of 1.2.
    warm_ps = ps_misc.tile([128, 128], F32, tag="misc", name="warm_ps")
    for _ in range(WARMUP):
        nc.tensor.matmul(warm_ps[0:32, :], warm[:, 0:32], warm[:, :],
                         start=True, stop=True)

    # ---------------- gathers (gpsimd queue; independent of attention) ---
    # g_all free layout: j (16) x e (64) x c (4); one indirect DMA per j.
    g_all = big.tile([128, NJ * C * EMB], BF, tag="g_all")
    for j in range(NJ):
        nc.gpsimd.indirect_dma_start(
            out=g_all[:, C * EMB * j : C * EMB * (j + 1)],
            out_offset=None,
            in_=io["w3T"][:, :],
            in_offset=IndirectOffsetOnAxis(ap=qidx[:, j : j + 1], axis=0),
        )

    fw = big.tile([128, NJ * C], BF, tag="fw")
    out_all = big.tile([128, NJ * EMB], BF, tag="out_all")

    # ---------------- per-batch stages ------------------------------------
    qs_l, ks_l, v_l = {}, {}, {}

    def stage_qk(b):
        ceT = cet[b]
        qs_ps = ps_misc.tile([128, S], F32, tag="misc", name="qs_ps")
        for h in range(HEADS):
            nc.tensor.matmul(
                qs_ps[32 * h : 32 * (h + 1), :],
                wqkv[:, 32 * h : 32 * (h + 1)],
                ceT[:, :],
                start=True, stop=True,
                tile_position=(0, 32 * h),
            )
        qs = sb.tile([128, S], BF, tag="qs", name="qs")
        nc.vector.tensor_copy(qs[:, :], qs_ps[:, :])
        qs_l[b] = qs
        ks_ps = ps_misc.tile([128, S], F32, tag="misc", name="ks_ps")
        for h in range(HEADS):
            nc.tensor.matmul(
                ks_ps[32 * h : 32 * (h + 1), :],
                wqkv[:, 128 + 32 * h : 128 + 32 * (h + 1)],
                ceT[:, :],
                start=True, stop=True,
                tile_position=(0, 32 * h),
            )
        ks = sb.tile([128, S], BF, tag="ks", name="ks")
        # b=0 is ramp-critical: evac ks on the (still idle) ACT engine so the
        # qs/ks casts run in parallel; copy shares the Exp activation table.
        if b == 0:
            nc.scalar.copy(ks[:, :], ks_ps[:, :])
        else:
            nc.vector.tensor_copy(ks[:, :], ks_ps[:, :])
        ks_l[b] = ks

    def stage_v(b):
        ceT = cet[b]
        v_ps = ps_misc.tile([128, S], F32, tag="misc", name="v_ps")
        for cc in range(4):
            nc.tensor.matmul(
                v_ps[:, 128 * cc : 128 * (cc + 1)],
                ceT[:, 128 * cc : 128 * (cc + 1)],
                wqkv[:, 256:384],
                start=True, stop=True,
            )
        v_sp = sb.tile([128, S], BF, tag="v_sp", name="v_sp")
        nc.vector.tensor_copy(v_sp[:, :], v_ps[:, :])
        v_l[b] = v_sp

    # ---- scores/exp/AV software pipeline --------------------------------
    def qk_wave(b, cc):
        qs, ks = qs_l[b], ks_l[b]
        ets = []
        tiles = []
        for hh in range(2):  # head halves
            sc_ps = ps_scores.tile([128, 2 * S], F32, tag="scores")
            for hi in range(2):
                h = 2 * hh + hi
                if MERGED_QK:
                    nc.tensor.matmul(
                        sc_ps[:, S * hi : S * (hi + 1)],
                        ks[32 * h : 32 * h + HD, 128 * cc : 128 * (cc + 1)],
                        qs[32 * h : 32 * h + HD, :],
                        start=True,
                        stop=True,
                        tile_position=(32 * h, 0),
                    )
                else:
                    for jj in range(4):
                        nc.tensor.matmul(
                            sc_ps[32 * jj : 32 * (jj + 1), S * hi : S * (hi + 1)],
                            ks[32 * h : 32 * h + HD,
                               128 * cc + 32 * jj : 128 * cc + 32 * (jj + 1)],
                            qs[32 * h : 32 * h + HD, :],
                            start=True,
                            stop=True,
                            tile_position=(32 * h, 32 * jj),
                        )
            tiles.append(sc_ps)
        for sc_ps in tiles:
            et = expp.tile([128, 2 * S], BF, tag="expT")
            nc.scalar.activation(et[:, :], sc_ps[:, :], ACTF.Exp, scale=SCALE)
            ets.append(et)
        return ets

    def av_wave(b, cc, avt_ps, sums_ps, ets):
        for h in range(HEADS):
            mv = ets[h // 2][:, S * (h % 2) : S * (h % 2 + 1)]
            nc.tensor.matmul(
                avt_ps[32 * h : 32 * (h + 1), :],
                v_l[b][:, 128 * cc + 32 * h : 128 * cc + 32 * (h + 1)],
                mv,
                start=(cc == 0),
                stop=(cc == 3),
                tile_position=(0, 32 * h),
                skip_group_check=True,
            )
        for h in range(HEADS):
            mv = ets[h // 2][:, S * (h % 2) : S * (h % 2 + 1)]
            nc.tensor.matmul(
                sums_ps[32 * h : 32 * (h + 1), :],
                ones_bf[:, :],
                mv,
                start=(cc == 0),
                stop=(cc == 3),
                tile_position=(0, 32 * h),
                skip_group_check=True,
            )

    def post_batch(b, avt_ps, sums_ps):
        rec = sb.tile([128, S], F32, tag="rec")
        nc.vector.reciprocal_approx_fast(rec[:, :], sums_ps[:, :])
        normT = sb.tile([128, S], BF, tag="normT")
        nc.vector.tensor_tensor(normT[:, :], avt_ps[:, :], rec[:, :], op=ALU.mult)
        z_ps = ps_misc.tile([128, 4 * C], F32, tag="misc", name="z_ps")
        for cc in range(4):
            nc.tensor.matmul(
                z_ps[:, C * cc : C * (cc + 1)],
                normT[:, 128 * cc : 128 * (cc + 1)],
                wzs[:, :],
                start=True,
                stop=True,
            )
        # u = 2 - sigmoid(z+bz) = (2+t)/(1+t) = 1 + 1/(1+t), t = e^z * e^bz
        tb = sb.tile([128, 4 * C], F32, tag="tb")
        nc.scalar.activation(tb[:, :], z_ps[:, :], ACTF.Exp)
        if not skip_ebc:
            tbe = sb.tile([128, 4 * C], F32, tag="tbe")
            nc.vector.tensor_tensor(tbe[:, :], tb[:, :], ebc[:, :], op=ALU.mult)
            tb = tbe
        ab = sb.tile([128, 4 * C], F32, tag="ab")
        nc.vector.tensor_scalar_add(ab[:, :], tb[:, :], 1.0)
        rb = sb.tile([128, 4 * C], F32, tag="rb")
        nc.vector.reciprocal_approx_fast(rb[:, :], ab[:, :])
        # fw = u * sharp = (1 + rb) * sharp  (0.5/head-mean folded in sharp)
        nc.vector.scalar_tensor_tensor(
            fw[:, 16 * b : 16 * (b + 1)],
            rb[:, :],
            1.0,
            sharp[:, 16 * b : 16 * (b + 1)],
            op0=ALU.add,
            op1=ALU.mult,
        )

        # final gather-contract: out[j,e] = sum_c g[j,e,c] * fw[j,c]
        # split across DVE and Pool for the last batch to shorten the tail.
        def gview(j0, j1):
            gsl = g_all[:, C * EMB * (NB * b + j0) : C * EMB * (NB * b + j1)]
            gv = gsl.rearrange("p (j e c) -> p j e c", e=EMB, c=C)
            fv = fw[:, 16 * b + C * j0 : 16 * b + C * j1].rearrange(
                "p (j c) -> p j c", c=C
            )[:, :, None, :].to_broadcast([128, j1 - j0, EMB, C])
            return gv, fv

        def reduce_half(j0, j1):
            gv, _ = gview(j0, j1)
            osl = out_all[:, EMB * (NB * b + j0) : EMB * (NB * b + j1)]
            nc.vector.tensor_reduce(
                osl.rearrange("p (j e) -> p j e", e=EMB),
                gv,
                axis=mybir.AxisListType.X,
                op=ALU.add,
            )

        def store(j0, j1):
            nc.sync.dma_start(
                out=io["out"][:, NB * b + j0 : NB * b + j1, :],
                in_=out_all[
                    :, EMB * (NB * b + j0) : EMB * (NB * b + j1)
                ].rearrange("p (j e) -> p j e", e=EMB),
            )

        if b == NB - 1:
            gv0, fv0 = gview(0, 2)
            nc.vector.tensor_tensor(gv0, gv0, fv0, op=ALU.mult)
            gv1, fv1 = gview(2, 4)
            nc.gpsimd.tensor_tensor(gv1, gv1, fv1, op=ALU.mult)
            reduce_half(0, 2)
            store(0, 2)
            reduce_half(2, 4)
            store(2, 4)
        else:
            gv0, fv0 = gview(0, 4)
            nc.vector.tensor_tensor(gv0, gv0, fv0, op=ALU.mult)
            reduce_half(0, 4)
            store(0, 4)

    ets_l = {b: [None] * 4 for b in range(NB)}
    av_tiles = {}

    def get_av(b):
        if b not in av_tiles:
            av_tiles[b] = (
                ps_av.tile([128, S], F32, tag="avt", name="avt_ps"),
                ps_sum.tile([128, S], F32, tag="sums", name="sums_ps"),
            )
        return av_tiles[b]

    stage_qk(0)
    prev = None  # (b, cc) whose AV wave is pending, 1 step behind
    for b in range(NB):
        for cc in range(4):
            ets_l[b][cc] = qk_wave(b, cc)
            if b == 0 and cc == 0:
                stage_v(0)
            if b + 1 < NB:
                if cc == 0:
                    stage_qk(b + 1)
                elif cc == 1:
                    stage_v(b + 1)
            if prev is not None:
                pb, pcc = prev
                av_wave(pb, pcc, *get_av(pb), ets_l[pb][pcc])
                if pcc == 3:
                    post_batch(pb, *av_tiles[pb])
            prev = (b, cc)
    av_wave(NB - 1, 3, *get_av(NB - 1), ets_l[NB - 1][3])
    post_batch(NB - 1, *av_tiles[NB - 1])


# ======================= host side =======================

def _prep_weights(inp):
    """Pure host-side folding of the (small, replicated) parameters."""
    f32 = np.float32

    def bf(x):
        return np.ascontiguousarray(np.asarray(x, f32).astype(BF16))

    W_ctx = np.asarray(inp["W_ctx"], f32)
    W_in = np.asarray(inp["W_in"], f32)
    W_out = np.asarray(inp["W_out"], f32)
    W_sup = np.asarray(inp["W_sup"], f32)
    W_emb = np.asarray(inp["W_emb"], f32)
    b_ctx = np.asarray(inp["b_ctx"], f32)
    b_in = np.asarray(inp["b_in"], f32)
    b_out = np.asarray(inp["b_out"], f32)
    b_sup = np.asarray(inp["b_sup"], f32)
    temp = np.asarray(inp["temperature"], np.float64)

    w = {}

    # q/k/v projections folded with the ctx projection; bias via ones row.
    def spread(M, c):
        # M (32, 64) rows 8h+d, c (32,) -> (65, 128) cols 32h+d
        out = np.zeros((EMB + 1, 128), f32)
        for h in range(HEADS):
            out[0:EMB, 32 * h : 32 * h + HD] = M[HD * h : HD * (h + 1), :].T
            out[EMB, 32 * h : 32 * h + HD] = c[HD * h : HD * (h + 1)]
        return out

    parts = []
    for i in range(3):
        Wp = W_in[ATTN * i : ATTN * (i + 1)]
        bp = b_in[ATTN * i : ATTN * (i + 1)]
        parts.append(spread(Wp @ W_ctx, Wp @ b_ctx + bp))
    w["wqkv"] = bf(np.concatenate(parts, axis=1))  # (65, 384)

    # suppression: z = o @ (W_sup W_out)^T + (W_sup b_out + b_sup)
    Wz = W_sup @ W_out            # (4, 32)
    bz = W_sup @ b_out + b_sup    # (4,)
    wzs = np.zeros((128, C), f32)
    for h in range(HEADS):
        wzs[32 * h : 32 * h + HD, :] = Wz[:, HD * h : HD * (h + 1)].T
    ebc = np.broadcast_to(np.tile(np.exp(bz).astype(f32), 4)[None, :], (128, 16))

    # ordinal-softmax table: tbl[r, c] = 0.5 * mean_h softmax_c(bw(r,.)/T_h)
    kk = np.arange(C, dtype=np.float64)
    tbl = np.zeros((C, C), np.float64)
    for r in range(C):
        bw = np.clip(1.0 - np.abs(kk - r) / (C - 1), 0.0, None)
        sh = np.exp(bw[None, :] / temp[:, None])
        sh /= sh.sum(axis=1, keepdims=True)
        tbl[r] = 0.5 * sh.mean(axis=0)
    w["_tbl"] = tbl.astype(f32)
    w["_wzs"] = wzs
    w["_ebc"] = np.ascontiguousarray(ebc, dtype=f32)

    # gather table: w3T[q, 4e+c] = W_emb[e, c*Q+q]
    w["w3T"] = bf(W_emb.reshape(EMB, C, Q).transpose(2, 0, 1).reshape(Q, EMB * C))
    return w


def _spec():
    return {
        "wqkv": ((EMB + 1, 3 * 128), BF),
        "blob": ((128, 84), BF),
        "qidx": ((128, NJ), I32),
        "w3T": ((Q, C * EMB), BF),
        "ce": ((NB, EMB + 1, S), BF),
    }


def build_bass(skip_ebc: bool = False):
    nc = bacc.Bacc("TRN2", target_bir_lowering=False, debug=False)
    io = {}
    for name, (shape, dt) in _spec().items():
        io[name] = nc.dram_tensor(name, list(shape), dt, kind="ExternalInput").ap()
    io["out"] = nc.dram_tensor("out", [128, NJ, EMB], BF, kind="ExternalOutput").ap()
    with tile.TileContext(nc) as tc:
        build_kernel(nc, tc, io, skip_ebc)
    nc.compile()
    return nc


def skip_ebc_for(inputs) -> bool:
    """True when W_sup@b_out + b_sup == 0, so e^bz == 1 can be elided."""
    W_sup = np.asarray(inputs["W_sup"], np.float64)
    b_out = np.asarray(inputs["b_out"], np.float64)
    b_sup = np.asarray(inputs["b_sup"], np.float64)
    return bool(np.all(W_sup @ b_out + b_sup == 0.0))


def make_in_maps(inputs):
    inp = dict(inputs)
    w = _prep_weights(inp)
    tbl, wzs, ebc = w.pop("_tbl"), w.pop("_wzs"), w.pop("_ebc")
    q_idx = np.asarray(inp["q_idx"]).astype(np.int32)
    r_data = np.asarray(inp["r_data"]).astype(np.int64)
    ce = np.asarray(inp["context_embedding"], np.float32)

    in_maps = []
    for k in range(NCORES):
        m = dict(w)
        qs = q_idx[NB * k : NB * (k + 1)]          # (4,512)
        rs = r_data[NB * k : NB * (k + 1)]
        # token-tile layout: [p, j] with j = 4*b + cc, s = 128*cc + p
        m["qidx"] = np.ascontiguousarray(
            qs.reshape(NB, 4, 128).transpose(2, 0, 1).reshape(128, NJ)
        )
        # sharp values per token: (128, 64) = (p, j*4+c)
        sharp = tbl[rs.reshape(NB, 4, 128)]        # (b, cc, p, c)
        sharp = sharp.transpose(2, 0, 1, 3).reshape(128, NJ * C)
        blob = np.zeros((128, 84), np.float32)
        blob[:, 0:64] = sharp
        blob[:, 64:68] = wzs
        blob[:, 68:84] = ebc
        m["blob"] = blob.astype(BF16)
        # ceT with ones row: (NB, 65, 512)
        cek = ce[NB * k : NB * (k + 1)]            # (4, 512, 64)
        cet = np.ones((NB, EMB + 1, S), np.float32)
        cet[:, 0:EMB, :] = cek.transpose(0, 2, 1)
        m["ce"] = cet.astype(BF16)
        in_maps.append(m)
    return in_maps


_NC_CACHE = {}


def postprocess(res, inputs) -> np.ndarray:
    b_emb = np.asarray(inputs["b_emb"], np.float32)
    outs = []
    for k in range(NCORES):
        o = np.asarray(res.results[k]["out"]).astype(np.float32)  # (128,16,64)
        o = o.reshape(128, NB, 4, EMB).transpose(1, 2, 0, 3).reshape(NB, S, EMB)
        outs.append(o)
    out = np.concatenate(outs, axis=0) + b_emb[None, None, :]
    return out.astype(np.float32)


def kernel(**inputs) -> np.ndarray:
    key = skip_ebc_for(inputs)
    if key not in _NC_CACHE:
        _NC_CACHE[key] = build_bass(skip_ebc=key)
    nc = _NC_CACHE[key]
    in_maps = make_in_maps(inputs)
    res = run_bass_kernel_spmd(nc, in_maps, core_ids=list(range(NCORES)))
    return postprocess(res, inputs)
